# revision 40
# baseline (speedup 1.0000x reference)
"""Trainium2 Bass kernel for nn_CrossMed4 (CrossMed-style GRU-over-GRU model).

v3: fully gather-free monitor lookups (one-hot PE matmuls), all-bf16.

The SWDGE dma_gather ucode is a hard serial floor (~8.4ns/row, one queue),
so monitor embedding lookups avoid it entirely:
- Value tables (vocab<=200, 2 vocab tiles) live in SBUF; per 128-token rank
  a one-hot OH[vocab_row, tok] is built on DVE (is_equal of host-broadcast
  ids vs an iota column) and accumulating PE matmuls OH.T @ table produce
  the value rows [tok, d] in PSUM.
- Item tables are binned ON HOST: each chunk's 3072 tokens are sorted by
  128-row vocab window, packed into ranks with a fixed rank->window plan, so
  each rank needs exactly ONE one-hot matmul. Bin overflow is evicted to one
  "spare" rank per chunk whose item rows come from a single up-front SWDGE
  gather per stream.
- Binning permutes tokens, so the sum-over-codes uses a host-streamed 0/1
  matrix S [tok, group] per rank (PE matmul, accumulated over ranks into one
  PSUM group tile).
- Both GRU levels in transposed layout (H^T [D=128, batch*keys]); bf16
  weights (fast-weight-load); steps split into two key-groups to pipeline
  PE/ACT/DVE across the serial recurrence. The weight/age visit-GRU keys
  (5,6) run before the monitor phase since they don't depend on it.
"""
import numpy as np
import ml_dtypes

try:
    import concourse.bass as bass  # noqa: F401
except ImportError:
    import sys
    sys.path.insert(0, "/opt/trn_rl_repo")

import concourse.bacc as bacc
import concourse.bass as bass
import concourse.mybir as mybir
import concourse.tile as tile
from concourse.bass_utils import run_bass_kernel_spmd

F32 = mybir.dt.float32
BF16 = mybir.dt.bfloat16
FP16 = mybir.dt.float16
F8 = mybir.dt.float8e4
I16 = mybir.dt.int16
BF = ml_dtypes.bfloat16
F8NP = ml_dtypes.float8_e4m3
AF = mybir.ActivationFunctionType
OP = mybir.AluOpType

B, V, M, L, D, OUT = 16, 16, 32, 24, 128, 193
VOCAB = {"cond": 5000, "proc": 2000, "drug": 600, "lab_item": 700,
         "lab_value": 200, "inj_item": 400, "inj_value": 200}
NCORES = 8
BL = B // NCORES            # 2 patients per core
NBV = BL * V                # 32 visit groups
TCH = 4                     # monitor steps per chunk
NCHUNK = M // TCH           # 8
GC = NBV * TCH              # 128 groups per chunk
NTOK = GC * L               # 3072 real tokens per chunk per stream
VRANKS = (NBV + 4) // 5     # 7
VIDX = VRANKS * 128         # 896

# rank -> item vocab window plan (binned ranks; one extra spare rank each)
RPLAN = {"lab": [0] * 5 + [1] * 5 + [2] * 5 + [3] * 5 + [4] * 5 + [5] * 3,
         "inj": [0] * 8 + [1] * 8 + [2] * 8 + [3] * 2}
NWIN = {"lab": 6, "inj": 4}
NBIN = {s: len(RPLAN[s]) for s in ("lab", "inj")}      # 28 / 26
NRANK = {s: NBIN[s] + 1 for s in ("lab", "inj")}       # incl spare: 29 / 27
CSW = {s: NRANK[s] * 128 for s in ("lab", "inj")}      # chunk-stream width
ITEM_OF = {"lab": "lab_item", "inj": "inj_item"}
VAL_OF = {"lab": "lab_value", "inj": "inj_value"}
# chunks whose VALUE rows come via SWDGE gather (on the otherwise-idle
# GpSimd) instead of one-hot matmuls, relieving PE/ACT/DVE
CONV = ((6, "lab"), (6, "inj"), (7, "lab"), (7, "inj"))
# iota const layout: cols 0-1 value tiles, 2..7 lab windows, 8..11 inj windows
IOTA_I0 = {"lab": 2, "inj": 8}

WSHAPES = (("mwhhT", [128, 1920]), ("mwihT34", [128, 768]),
           ("mxgb34", [1, 768]), ("mwihT012", [128, 1152]),
           ("mxgb012", [1, 1152]), ("vwhhT", [128, 2688]),
           ("vwihT04", [128, 1920]), ("vxgb04", [1, 1920]),
           ("vxg56u", [1, 768]), ("vxg56c", [1, 768]),
           ("S5", [128, 5]), ("ones", [1, 224]),
           ("wa", [1, 64]), ("fcw", [128, 7 * OUT]), ("fcb", [1, OUT]),
           ("vtab_lab", [128, 256]), ("vtab_inj", [128, 256]),
           ("itab_lab", [128, NWIN["lab"] * 128]),
           ("itab_inj", [128, NWIN["inj"] * 128]))
WSHAPES_F32 = (("bhn_bc", [128, 160]), ("vbhn_bc", [128, 14]),
               ("iotas", [128, 12]))


# --------------------------------------------------------------------------
# host-side index / weight packing
# --------------------------------------------------------------------------

def _wrap_idx(flat):
    # token i lives at [i % 16, i // 16]; the gather ucode's Q7 cores each
    # read their own 16-partition band, so replicate to all 8 bands.
    n = flat.shape[0]
    return np.tile(flat.reshape(n // 16, 16).T, (8, 1)).astype(np.int16)


def _build_visit_idx(tok):
    flat = np.zeros(VIDX, dtype=np.int64)
    t = np.asarray(tok)
    for r in range(VRANKS):
        for j in range(5):
            slot = 5 * r + j
            if slot >= NBV:
                continue
            b, v = divmod(slot, V)
            flat[r * 128 + j * 24: r * 128 + j * 24 + 24] = t[b, v, :]
    return _wrap_idx(flat)


def _bin_chunk(ids_i, ids_v, grp, sname):
    """Bin one chunk-stream's tokens by item vocab window.

    ids_i/ids_v/grp: [NTOK] arrays (item id, value id, group id 0..127).
    Returns (ibc_row, vbc_row, S, spare_idx) where rows are [CSW] and
    S is [128, CSW] one-hot token->group.
    """
    nrank, nbin = NRANK[sname], NBIN[sname]
    W = CSW[sname]
    win = ids_i >> 7
    order = np.lexsort((grp, win))
    ids_i, ids_v, grp, win = (a[order] for a in (ids_i, ids_v, grp, win))
    plan = np.asarray(RPLAN[sname])
    ib = np.full(W, -1, dtype=np.int64)  # ids reduced by window base
    vb = np.zeros(W, dtype=np.int64)
    gr = np.full(W, -1, dtype=np.int64)
    spare = []
    pos = 0
    for w in range(NWIN[sname]):
        ranks = np.nonzero(plan == w)[0]
        cap = len(ranks) * 128
        base = ranks[0] * 128
        nw = int(np.searchsorted(win, w + 1)) - pos
        take = min(nw, cap)
        sl = slice(pos, pos + take)
        ib[base:base + take] = ids_i[sl] - 128 * w
        vb[base:base + take] = ids_v[sl]
        gr[base:base + take] = grp[sl]
        if nw > take:  # bin overflow -> spare rank
            spare.extend(range(pos + take, pos + nw))
        pos += nw
    assert len(spare) <= 128, f"spare overflow: {len(spare)}"
    sb = nbin * 128
    sp_idx = np.zeros(128, dtype=np.int64)
    if spare:
        sp = np.asarray(spare)
        n = len(sp)
        sp_idx[:n] = ids_i[sp]
        vb[sb:sb + n] = ids_v[sp]
        gr[sb:sb + n] = grp[sp]
    S = np.zeros((128, W), dtype=np.float32)
    cols = np.arange(W)
    real = gr >= 0
    S[cols[real] % 128, (cols[real] // 128) * 128 + gr[real]] = 1.0
    # S layout fix: S[p, r*128 + g]; p is the slot-within-rank
    return ib, vb, S, sp_idx


def _prep_shared(inputs):
    """Weight repacking shared by all cores (pure layout transforms)."""
    f = {k: np.asarray(v, dtype=np.float32) for k, v in inputs.items()
         if not k.startswith("tok_")}
    sh = {}
    mwih, mwhh = f["mgru_wih"], f["mgru_whh"]
    mbih, mbhh = f["mgru_bih"], f["mgru_bhh"]
    vwih, vwhh = f["vgru_wih"], f["vgru_whh"]
    vbih, vbhh = f["vgru_bih"], f["vgru_bhh"]

    def packT(w_keys):  # [K, 3D, D] -> [128, K*3*128], col (k*3+gi)*128+gu
        k = w_keys.shape[0]
        out = np.zeros((128, k * 3 * 128), dtype=np.float32)
        for ki in range(k):
            for gi in range(3):
                out[:, (ki * 3 + gi) * 128:(ki * 3 + gi + 1) * 128] = \
                    w_keys[ki, gi * 128:(gi + 1) * 128, :].T
        return out

    def pack_xgb(bih, bhh, keys):  # -> [1, len(keys)*384]
        rows = []
        for k in keys:
            b = bih[k].copy()
            b[:2 * D] += bhh[k][:2 * D]
            rows.append(b)
        return np.concatenate(rows)[None, :]

    sh["mwhhT"] = packT(mwhh)
    sh["mwihT34"] = packT(mwih[3:5])
    sh["mxgb34"] = pack_xgb(mbih, mbhh, [3, 4])
    sh["mwihT012"] = packT(mwih[0:3])
    sh["mxgb012"] = pack_xgb(mbih, mbhh, [0, 1, 2])
    sh["bhn_bc"] = np.repeat(mbhh[:, 2 * D:].T, NBV, axis=1).astype(np.float32)
    sh["vwhhT"] = packT(vwhh)
    sh["vwihT04"] = packT(vwih[0:5])
    sh["vxgb04"] = pack_xgb(vbih, vbhh, [0, 1, 2, 3, 4])
    u_rows, c_rows = [], []
    for k in (5, 6):
        u_rows.append(vwih[k] @ f["info_w"][k - 5])
        cv = vwih[k] @ f["info_b"][k - 5] + vbih[k]
        cv[:2 * D] += vbhh[k][:2 * D]
        c_rows.append(cv)
    sh["vxg56u"] = np.concatenate(u_rows)[None, :]
    sh["vxg56c"] = np.concatenate(c_rows)[None, :]
    sh["vbhn_bc"] = np.repeat(vbhh[:, 2 * D:].T, BL, axis=1).astype(np.float32)
    s5 = np.zeros((128, 5), dtype=np.float32)
    for j in range(5):
        s5[j * 24:(j + 1) * 24, j] = 1.0
    sh["S5"] = s5
    sh["ones"] = np.ones((1, 224), dtype=np.float32)
    fcw = np.zeros((128, 7 * OUT), dtype=np.float32)
    for k in range(7):
        fcw[:, k * OUT:(k + 1) * OUT] = f["fc_w"][k * D:(k + 1) * D, :]
    sh["fcw"] = fcw
    sh["fcb"] = f["fc_b"][None, :]
    for s in ("lab", "inj"):
        tabf = f["emb_" + VAL_OF[s]]
        vt = np.zeros((128, 256), dtype=np.float32)
        vt[:, 0:128] = tabf[0:128]
        vt[:tabf.shape[0] - 128, 128:256] = tabf[128:]
        sh["vtab_" + s] = vt
        tabi = f["emb_" + ITEM_OF[s]]
        it = np.zeros((128, NWIN[s] * 128), dtype=np.float32)
        for w in range(NWIN[s]):
            rows = tabi[w * 128:(w + 1) * 128]
            it[:rows.shape[0], w * 128:w * 128 + 128] = rows
        sh["itab_" + s] = it
    iv = np.zeros((128, 12), dtype=np.float32)
    p = np.arange(128)
    iv[:, 0] = p
    iv[:, 1] = p + 128
    for s in ("lab", "inj"):
        for w in range(NWIN[s]):
            iv[:, IOTA_I0[s] + w] = p + 128 * w
    sh["iotas"] = iv
    out = {}
    for n, _ in WSHAPES:
        if n == "wa":
            continue
        out[n] = sh[n].astype(BF)
    for n, _ in WSHAPES_F32:
        out[n] = sh[n].astype(np.float32)
    for name in ("cond", "proc", "drug", "lab_item", "inj_item",
                 "lab_value", "inj_value"):
        out["emb_" + name] = f["emb_" + name].astype(BF)
    return out


def _prep_core(inputs, shared, core):
    b0 = core * BL
    m = dict(shared)
    for name in ("cond", "proc", "drug"):
        m["idx_" + name] = _build_visit_idx(
            np.asarray(inputs["tok_" + name])[b0:b0 + BL])
    for sname in ("lab", "inj"):
        ti = np.asarray(inputs["tok_" + ITEM_OF[sname]])[b0:b0 + BL] \
            .reshape(NBV, M, L)
        tv = np.asarray(inputs["tok_" + VAL_OF[sname]])[b0:b0 + BL] \
            .reshape(NBV, M, L)
        sp_flat = np.zeros(NCHUNK * 128, dtype=np.int64)
        for c in range(NCHUNK):
            msl = slice(c * TCH, (c + 1) * TCH)
            ids_i = ti[:, msl, :].transpose(1, 0, 2).reshape(-1)
            ids_v = tv[:, msl, :].transpose(1, 0, 2).reshape(-1)
            grp = (np.arange(TCH)[:, None, None] * NBV +
                   np.arange(NBV)[None, :, None] +
                   0 * np.arange(L)[None, None, :]).reshape(-1)
            ib, vb, S, sp = _bin_chunk(ids_i, ids_v, grp, sname)
            m[f"ibc_{sname}_{c}"] = np.ascontiguousarray(
                np.broadcast_to(ib.astype(BF)[None, :], (128, CSW[sname])))
            if (c, sname) in CONV:
                m[f"vidx_{sname}_{c}"] = _wrap_idx(vb)
            else:
                m[f"vbc_{sname}_{c}"] = np.ascontiguousarray(
                    np.broadcast_to(vb.astype(BF)[None, :], (128, CSW[sname])))
            m[f"smat_{sname}_{c}"] = S.astype(F8NP)
            sp_flat[c * 128:(c + 1) * 128] = sp
        m[f"spidx_{sname}"] = _wrap_idx(sp_flat)
    wa = np.zeros((1, 64), dtype=np.float32)
    wa[0, :NBV] = np.asarray(inputs["weight"], np.float32)[b0:b0 + BL].reshape(NBV)
    wa[0, NBV:] = np.asarray(inputs["age"], np.float32)[b0:b0 + BL].reshape(NBV)
    m["wa"] = wa.astype(BF)
    return m


# --------------------------------------------------------------------------
# device program
# --------------------------------------------------------------------------

def build_nc():
    nc = bacc.Bacc("TRN2", target_bir_lowering=False, debug=False,
                   num_devices=NCORES)
    dt = {}
    for name in ("cond", "proc", "drug", "lab_item", "inj_item",
                 "lab_value", "inj_value"):
        dt["emb_" + name] = nc.dram_tensor("emb_" + name, [VOCAB[name], D],
                                           BF16, kind="ExternalInput")
    for name in ("cond", "proc", "drug"):
        dt["idx_" + name] = nc.dram_tensor("idx_" + name, [128, VIDX // 16],
                                           I16, kind="ExternalInput")
    for s in ("lab", "inj"):
        dt[f"spidx_{s}"] = nc.dram_tensor(f"spidx_{s}", [128, NCHUNK * 8],
                                          I16, kind="ExternalInput")
        for c in range(NCHUNK):
            dt[f"ibc_{s}_{c}"] = nc.dram_tensor(f"ibc_{s}_{c}",
                                                [128, CSW[s]], BF16,
                                                kind="ExternalInput")
            if (c, s) in CONV:
                dt[f"vidx_{s}_{c}"] = nc.dram_tensor(
                    f"vidx_{s}_{c}", [128, CSW[s] // 16], I16,
                    kind="ExternalInput")
            else:
                dt[f"vbc_{s}_{c}"] = nc.dram_tensor(f"vbc_{s}_{c}",
                                                    [128, CSW[s]], BF16,
                                                    kind="ExternalInput")
            dt[f"smat_{s}_{c}"] = nc.dram_tensor(f"smat_{s}_{c}",
                                                 [128, CSW[s]], F8,
                                                 kind="ExternalInput")
    for name, shape in WSHAPES:
        dt[name] = nc.dram_tensor(name, shape, BF16, kind="ExternalInput")
    for name, shape in WSHAPES_F32:
        dt[name] = nc.dram_tensor(name, shape, F32, kind="ExternalInput")
    out_logits = nc.dram_tensor("logits", [BL, OUT], F32, kind="ExternalOutput")

    with tile.TileContext(nc) as tc:
        _program(nc, tc, dt, out_logits)
    nc.compile()
    return nc


def _program(nc, tc, dt, out_logits):
    import contextlib
    ctx = contextlib.ExitStack()
    with ctx:
        cpool = ctx.enter_context(tc.tile_pool(name="const", bufs=1))
        gpool = ctx.enter_context(tc.tile_pool(name="gather", bufs=1))
        vbpool = ctx.enter_context(tc.tile_pool(name="vb", bufs=2))
        ohpool = ctx.enter_context(tc.tile_pool(name="oh", bufs=2))
        prpool = ctx.enter_context(tc.tile_pool(name="pr", bufs=2))
        spool = ctx.enter_context(tc.tile_pool(name="work", bufs=2))
        xgpool = ctx.enter_context(tc.tile_pool(name="xg34", bufs=3))
        hpool = ctx.enter_context(tc.tile_pool(name="h", bufs=2))
        p2 = ctx.enter_context(tc.tile_pool(name="psum2", bufs=2,
                                            space="PSUM"))
        p1 = ctx.enter_context(tc.tile_pool(name="psum1", bufs=1,
                                            space="PSUM"))

        # ---- load constants to SBUF; critical-path ones first, then chunk-0
        # stream data, then the rest (the sync queue issues in order).
        shapes = dict(WSHAPES)
        shapes_f32 = dict(WSHAPES_F32)
        cb = {}

        def _load(name):
            if name in shapes_f32:
                t = cpool.tile(shapes_f32[name], F32, tag=name)
            else:
                t = cpool.tile(shapes[name], BF16, tag=name)
            nc.sync.dma_start(t[:], dt[name].ap())
            cb[name] = t

        idx = {}
        for name in ("cond", "proc", "drug"):
            t = cpool.tile([128, VIDX // 16], I16, tag="idx_" + name)
            nc.sync.dma_start(t[:], dt["idx_" + name].ap())
            idx[name] = t
        for name in ("iotas", "vtab_lab", "vtab_inj", "itab_lab", "itab_inj",
                     "S5", "ones", "mwihT012", "mxgb012", "mwhhT", "bhn_bc",
                     "mwihT34", "mxgb34", "vxg56u", "vxg56c", "wa", "vwhhT",
                     "vbhn_bc"):
            _load(name)

        # chunk-stream tile prefetch (vb/ib/st for chunk c)
        stream_tiles = {}

        def _prefetch(c):
            if c >= NCHUNK:
                return
            for sname in ("lab", "inj"):
                W = CSW[sname]
                if (c, sname) in CONV:
                    vb = None
                else:
                    vb = vbpool.tile([128, W], BF16, tag="vb_" + sname)
                    nc.sync.dma_start(vb[:], dt[f"vbc_{sname}_{c}"].ap())
                ib = vbpool.tile([128, W], BF16, tag="ib_" + sname)
                nc.sync.dma_start(ib[:], dt[f"ibc_{sname}_{c}"].ap())
                st = vbpool.tile([128, W], F8, tag="st_" + sname)
                nc.sync.dma_start(st[:], dt[f"smat_{sname}_{c}"].ap())
                stream_tiles[(c, sname)] = (vb, ib, st)

        _prefetch(0)
        spi = {}
        for s in ("lab", "inj"):
            t = cpool.tile([128, NCHUNK * 8], I16, tag="spidx_" + s)
            nc.sync.dma_start(t[:], dt[f"spidx_{s}"].ap())
            spi[s] = t
        for name in ("vwihT04", "vxgb04", "fcw", "fcb"):
            _load(name)

        S5 = cb["S5"]
        ones = cb["ones"]
        iotas = cb["iotas"]

        # ---- visit-level features eT[k] = [128, 32] bf16
        eT = {}
        for name in ("cond", "proc", "drug"):
            gt = gpool.tile([128, VIDX], BF16, tag="vgather")
            nc.gpsimd.dma_gather(
                gt[:].rearrange("p (r e) -> p r e", e=D),
                dt["emb_" + name].ap(), idx[name][:], VIDX, VIDX, D)
            pr = p1.tile([128, 128], F32, tag="red")
            for r in range(VRANKS):
                nc.tensor.matmul(pr[:, 5 * r:5 * r + 5],
                                 gt[:, r * D:(r + 1) * D], S5[:],
                                 start=True, stop=True)
            et = cpool.tile([128, NBV], BF16, tag="eT_" + name)
            nc.scalar.copy(et[:], pr[:, :NBV])
            eT[name] = et

        # ---- spare-rank item gathers (one per stream, all chunks)
        spg = {}
        for s in ("lab", "inj"):
            gt = cpool.tile([128, NCHUNK * 128], BF16, tag="spg_" + s)
            nc.gpsimd.dma_gather(
                gt[:].rearrange("p (r e) -> p r e", e=D),
                dt["emb_" + ITEM_OF[s]].ap(), spi[s][:],
                NCHUNK * 128, NCHUNK * 128, D)
            spg[s] = gt

        # ---- value-row gathers for the CONV chunks run just-in-time on the
        # otherwise-idle GpSimd, two chunks ahead of use
        convt = {}
        vix = {}
        for cc, s in CONV:
            t = cpool.tile([128, CSW[s] // 16], I16, tag=f"vidx_{s}_{cc}")
            nc.sync.dma_start(t[:], dt[f"vidx_{s}_{cc}"].ap())
            vix[(cc, s)] = t

        def _conv_gather(cc):
            for s in ("lab", "inj"):
                if (cc, s) not in CONV:
                    continue
                W = CSW[s]
                vt = vbpool.tile([128, W], BF16, tag="vt_" + s)
                for r0 in range(0, NRANK[s], 8):
                    nr = min(8, NRANK[s] - r0)
                    nc.gpsimd.dma_gather(
                        vt[:].rearrange("p (r e) -> p r e", e=D)
                        [:, r0:r0 + nr, :],
                        dt["emb_" + VAL_OF[s]].ap(),
                        vix[(cc, s)][:, r0 * 8:(r0 + nr) * 8],
                        nr * 128, nr * 128, D)
                convt[(cc, s)] = vt

        # ---- XGc for monitor keys 0-2: [128, 3*96], col gi*96 + k*32 + bv
        xgc = cpool.tile([128, 288], BF16, tag="xgc")
        for k, name in enumerate(("cond", "proc", "drug")):
            pk = p1.tile([128, 384], F32, tag="xg")
            for gi in range(3):
                off = (k * 3 + gi) * 128
                nc.tensor.matmul(pk[:, gi * 32:(gi + 1) * 32],
                                 cb["mwihT012"][:, off:off + 128],
                                 eT[name][:], start=True, stop=False)
                nc.tensor.matmul(pk[:, gi * 32:(gi + 1) * 32],
                                 cb["mxgb012"][0:1, off:off + 128],
                                 ones[0:1, 0:NBV], start=False, stop=True)
            nc.scalar.copy(
                xgc[:].rearrange("p (g c) -> p g c", g=3)[:, :, k * 32:(k + 1) * 32],
                pk[:, 0:96].rearrange("p (g c) -> p g c", g=3))

        # ---- visit GRU xg for keys 5,6 (weight/age)
        vxg = cpool.tile([128, 672], BF16, tag="vxg")  # col gi*224 + k*32 + bv
        for k in (5, 6):
            pk = p1.tile([128, 384], F32, tag="xg")
            for gi in range(3):
                off = ((k - 5) * 3 + gi) * 128
                nc.tensor.matmul(pk[:, gi * 32:(gi + 1) * 32],
                                 cb["vxg56u"][0:1, off:off + 128],
                                 cb["wa"][0:1, (k - 5) * 32:(k - 4) * 32],
                                 start=True, stop=False)
                nc.tensor.matmul(pk[:, gi * 32:(gi + 1) * 32],
                                 cb["vxg56c"][0:1, off:off + 128],
                                 ones[0:1, 0:NBV], start=False, stop=True)
            nc.scalar.copy(
                vxg[:].rearrange("p (g c) -> p g c", g=3)[:, :, k * 32:(k + 1) * 32],
                pk[:, 0:96].rearrange("p (g c) -> p g c", g=3))

        # ---- visit GRU keys 5,6 state (steps interleaved into chunk loop)
        vh56 = cpool.tile([128, 4], BF16, tag="vh56")
        vhp56 = hpool.tile([128, 4], BF16, tag="VH56")
        nc.vector.memset(vhp56[:], 0.0)

        # ---- monitor chunks + GRU
        h_prev = hpool.tile([128, 160], BF16, tag="H")
        nc.vector.memset(h_prev[:], 0.0)

        for c in range(NCHUNK):
            xg34 = xgpool.tile([128, 768], BF16, tag="xg34")
            _prefetch(c + 1)
            if c + 4 in [cc for cc, _ in CONV]:
                _conv_gather(c + 4)
            for k, sname in ((3, "lab"), (4, "inj")):
                W = CSW[sname]
                nbin, nrank = NBIN[sname], NRANK[sname]
                vb, ib, st = stream_tiles.pop((c, sname))
                conv = (c, sname) in CONV
                vtab, itab = cb["vtab_" + sname], cb["itab_" + sname]
                ohi = ohpool.tile([128, W], BF16, tag="ohi")
                if not conv:
                    oh0 = ohpool.tile([128, W], BF16, tag="oh0")
                    oh1 = ohpool.tile([128, W], BF16, tag="oh1")
                    nc.vector.tensor_scalar(oh0[:], vb[:], iotas[:, 0:1],
                                            None, OP.is_equal)
                    nc.vector.tensor_scalar(oh1[:], vb[:], iotas[:, 1:2],
                                            None, OP.is_equal)
                plan = RPLAN[sname]
                nc.vector.tensor_scalar(
                    ohi[:, 0:nbin * 128], ib[:, 0:nbin * 128],
                    iotas[:, 0:1], None, OP.is_equal)
                xsum = p1.tile([128, 128], F32, tag="red")
                s_pending = []
                for r0 in range(0, nrank, 4):
                    nr = min(4, nrank - r0)
                    wd = nr * 128
                    if conv:
                        vtile, vofs = convt[(c, sname)], r0 * 128
                    else:
                        pv = p2.tile([128, 512], F32, tag="pv")
                        for j in range(nr):
                            sl = slice((r0 + j) * 128, (r0 + j + 1) * 128)
                            nc.tensor.matmul(pv[:, j * 128:(j + 1) * 128],
                                             oh0[:, sl], vtab[:, 0:128],
                                             start=True, stop=False)
                            nc.tensor.matmul(pv[:, j * 128:(j + 1) * 128],
                                             oh1[:, sl], vtab[:, 128:256],
                                             start=False, stop=True)
                        vtile = prpool.tile([128, 512], BF16, tag="pvb")
                        nc.scalar.copy(vtile[:, 0:wd], pv[:, 0:wd])
                        vofs = 0
                    pt = prpool.tile([128, 512], BF16, tag="prod")
                    nbin_j = min(nr, nbin - r0)  # binned ranks in this group
                    if nbin_j > 0:
                        pi = p2.tile([128, 512], F32, tag="pv")
                        for j in range(nbin_j):
                            w = plan[r0 + j]
                            sl = slice((r0 + j) * 128, (r0 + j + 1) * 128)
                            nc.tensor.matmul(
                                pi[:, j * 128:(j + 1) * 128],
                                ohi[:, sl], itab[:, w * 128:(w + 1) * 128],
                                start=True, stop=True)
                        nc.vector.tensor_tensor(pt[:, 0:nbin_j * 128],
                                                pi[:, 0:nbin_j * 128],
                                                vtile[:, vofs:vofs + nbin_j * 128],
                                                op=OP.mult)
                    if nbin_j < nr:  # spare rank (always last)
                        j = nbin_j
                        nc.vector.tensor_tensor(
                            pt[:, j * 128:(j + 1) * 128],
                            spg[sname][:, c * 128:(c + 1) * 128],
                            vtile[:, vofs + j * 128:vofs + (j + 1) * 128],
                            op=OP.mult)
                    for args in s_pending:
                        nc.tensor.matmul(*args[:3], start=args[3], stop=args[4],
                                         skip_group_check=True)
                    s_pending = []
                    for j in range(nr):
                        r = r0 + j
                        s_pending.append(
                            (xsum[:], pt[:, j * 128:(j + 1) * 128],
                             st[:, r * 128:(r + 1) * 128],
                             r == 0, r == nrank - 1))
                for args in s_pending:
                    nc.tensor.matmul(*args[:3], start=args[3], stop=args[4],
                                     skip_group_check=True)
                xmT = spool.tile([128, GC], BF16, tag="xmT")
                nc.scalar.copy(xmT[:], xsum[:])
                pxg = p1.tile([128, 384], F32, tag="xg")
                for gi in range(3):
                    off = ((k - 3) * 3 + gi) * 128
                    nc.tensor.matmul(pxg[:, gi * 128:(gi + 1) * 128],
                                     cb["mwihT34"][:, off:off + 128],
                                     xmT[:], start=True, stop=False)
                    nc.tensor.matmul(pxg[:, gi * 128:(gi + 1) * 128],
                                     cb["mxgb34"][0:1, off:off + 128],
                                     ones[0:1, 0:128], start=False, stop=True)
                # copy into xg34: col gi*256 + mi*64 + (k-3)*32 + bv
                for gi in range(3):
                    nc.scalar.copy(
                        xg34[:].rearrange("p (g m kb) -> p g m kb", g=3, m=TCH)
                        [:, gi, :, (k - 3) * 32:(k - 2) * 32],
                        pxg[:].rearrange("p (g m b) -> p g m b", g=3, m=TCH)
                        [:, gi, :, :])

            for mi in range(TCH):
                prz = p2.tile([128, 320], F32, tag="prz")
                pn = p2.tile([128, 160], F32, tag="pn")
                rz = spool.tile([128, 320], BF16, tag="rz")
                npre = spool.tile([128, 160], BF16, tag="npre")
                nt = spool.tile([128, 160], BF16, tag="nt")
                t3 = spool.tile([128, 160], BF16, tag="t3")
                h_new = hpool.tile([128, 160], BF16, tag="H")
                # two key-groups: A = keys 0-2 (cols 0:96), B = keys 3-4
                for klo, khi in ((0, 3), (3, 5)):
                    lo, hi = klo * 32, khi * 32
                    if klo == 0:
                        nc.scalar.copy(
                            prz[:].rearrange("p (g c) -> p g c", g=2)[:, :, 0:96],
                            xgc[:].rearrange("p (g c) -> p g c", g=3)[:, 0:2, :])
                    else:
                        nc.scalar.copy(
                            prz[:].rearrange("p (g c) -> p g c", g=2)[:, :, 96:160],
                            xg34[:].rearrange("p (g c) -> p g c", g=3)
                            [:, 0:2, mi * 64:(mi + 1) * 64])
                    nc.scalar.copy(pn[:, lo:hi], cb["bhn_bc"][:, lo:hi])
                    for k in range(klo, khi):
                        hs = h_prev[:, k * 32:(k + 1) * 32]
                        for gi in range(2):
                            nc.tensor.matmul(
                                prz[:, gi * 160 + k * 32: gi * 160 + (k + 1) * 32],
                                cb["mwhhT"][:, (k * 3 + gi) * 128:(k * 3 + gi + 1) * 128],
                                hs, start=False, stop=True, skip_group_check=True)
                        nc.tensor.matmul(
                            pn[:, k * 32:(k + 1) * 32],
                            cb["mwhhT"][:, (k * 3 + 2) * 128:(k * 3 + 3) * 128],
                            hs, start=False, stop=True, skip_group_check=True)
                    nc.scalar.activation(
                        rz[:].rearrange("p (g c) -> p g c", g=2)[:, :, lo:hi],
                        prz[:].rearrange("p (g c) -> p g c", g=2)[:, :, lo:hi],
                        AF.Sigmoid)
                    nc.vector.tensor_tensor(npre[:, lo:hi], rz[:, lo:hi],
                                            pn[:, lo:hi], op=OP.mult)
                    if klo == 0:
                        nc.vector.tensor_tensor(
                            npre[:, lo:hi], npre[:, lo:hi],
                            xgc[:].rearrange("p (g c) -> p g c", g=3)[:, 2, :],
                            op=OP.add)
                    else:
                        nc.vector.tensor_tensor(
                            npre[:, lo:hi], npre[:, lo:hi],
                            xg34[:, 2 * 256 + mi * 64: 2 * 256 + (mi + 1) * 64],
                            op=OP.add)
                    nc.scalar.activation(nt[:, lo:hi], npre[:, lo:hi], AF.Tanh)
                    nc.vector.tensor_tensor(t3[:, lo:hi], h_prev[:, lo:hi],
                                            nt[:, lo:hi], op=OP.subtract)
                    nc.vector.tensor_tensor(t3[:, lo:hi], t3[:, lo:hi],
                                            rz[:, 160 + lo:160 + hi],
                                            op=OP.mult)
                    nc.vector.tensor_tensor(h_new[:, lo:hi], t3[:, lo:hi],
                                            nt[:, lo:hi], op=OP.add)
                h_prev = h_new

            # two steps of the keys-5/6 visit GRU, hidden in chunk slack
            # (p1 tags: red/xg are idle during the GRU-step phase)
            for v in (2 * c, 2 * c + 1):
                prz = p1.tile([128, 128], F32, tag="red")
                pn = p1.tile([128, 384], F32, tag="xg")
                rz = spool.tile([128, 28], BF16, tag="vrz")
                u = spool.tile([128, 14], BF16, tag="vu")
                nt = spool.tile([128, 14], BF16, tag="vnt")
                t3 = spool.tile([128, 14], BF16, tag="vt3")
                vh_new = vh56 if v == V - 1 else hpool.tile([128, 4], BF16,
                                                            tag="VH56")
                _vgru_sub(nc, cb, vxg, prz, pn, rz, u, nt, t3, vhp56, vh_new,
                          v, 5, 7, hofs=10)
                vhp56 = vh_new

        # ---- visit GRU xg for keys 0-4 (from final monitor h)
        for k in range(5):
            pk = p1.tile([128, 384], F32, tag="xg")
            for gi in range(3):
                off = (k * 3 + gi) * 128
                nc.tensor.matmul(pk[:, gi * 32:(gi + 1) * 32],
                                 cb["vwihT04"][:, off:off + 128],
                                 h_prev[:, k * 32:(k + 1) * 32],
                                 start=True, stop=False)
                nc.tensor.matmul(pk[:, gi * 32:(gi + 1) * 32],
                                 cb["vxgb04"][0:1, off:off + 128],
                                 ones[0:1, 0:NBV], start=False, stop=True)
            nc.scalar.copy(
                vxg[:].rearrange("p (g c) -> p g c", g=3)[:, :, k * 32:(k + 1) * 32],
                pk[:, 0:96].rearrange("p (g c) -> p g c", g=3))

        # ---- visit GRU keys 0-4 (16 steps)
        vh_prev = hpool.tile([128, 10], BF16, tag="VH")
        nc.vector.memset(vh_prev[:], 0.0)
        for v in range(V):
            prz = p2.tile([128, 320], F32, tag="prz")
            pn = p2.tile([128, 160], F32, tag="pn")
            rz = spool.tile([128, 28], BF16, tag="vrz")
            u = spool.tile([128, 14], BF16, tag="vu")
            nt = spool.tile([128, 14], BF16, tag="vnt")
            t3 = spool.tile([128, 14], BF16, tag="vt3")
            vh_new = hpool.tile([128, 10], BF16, tag="VH")
            for klo, khi in ((0, 2), (2, 5)):
                _vgru_sub(nc, cb, vxg, prz, pn, rz, u, nt, t3, vh_prev,
                          vh_new, v, klo, khi, hofs=0)
            vh_prev = vh_new

        # ---- FC head
        rl = spool.tile([128, 14], BF16, tag="rl")
        nc.scalar.activation(rl[:, 0:10], vh_prev[:], AF.Relu)
        nc.scalar.activation(rl[:, 10:14], vh56[:], AF.Relu)
        pfc = p1.tile([BL, 384], F32, tag="xg")
        for k in range(7):
            nc.tensor.matmul(pfc[:, 0:OUT], rl[:, k * 2:(k + 1) * 2],
                             cb["fcw"][:, k * OUT:(k + 1) * OUT],
                             start=(k == 0), stop=False)
        nc.tensor.matmul(pfc[:, 0:OUT], ones[0:1, 0:BL], cb["fcb"][0:1, :],
                         start=False, stop=True)
        lg = spool.tile([BL, OUT], F32, tag="lg")
        nc.scalar.copy(lg[:], pfc[:, 0:OUT])
        nc.sync.dma_start(out_logits.ap(), lg[:])


def _vgru_sub(nc, cb, vxg, prz, pn, rz, u, nt, t3, vh_prev, vh_new,
              v, klo, khi, hofs):
    """One key-subrange of one visit-GRU step.

    vh_prev/vh_new are [128, 2*(khi-klo...)] slices indexed relative to hofs:
    h column for key k lives at (k*2 - hofs*2//...) -- callers pass tiles
    whose col 0 corresponds to key hofs//2... simplified: vh tiles hold keys
    [hofs/2, ...) with col (k - hofs_key)*2.
    """
    kofs = hofs // 2  # first key held in the vh tiles
    lo, hi = klo * 2, khi * 2
    nc.vector.tensor_scalar(
        prz[:, 0:28].rearrange("p (g c) -> p g c", g=2)[:, :, lo:hi],
        vxg[:].rearrange("p (g k b v2) -> p g k b v2", g=3, k=7, b=BL)
        [:, 0:2, klo:khi, :, v], 0.0, None, OP.add)
    nc.vector.tensor_scalar(pn[:, lo:hi], cb["vbhn_bc"][:, lo:hi],
                            0.0, None, OP.add)
    for k in range(klo, khi):
        hs = vh_prev[:, (k - kofs) * 2:(k - kofs + 1) * 2]
        for gi in range(2):
            nc.tensor.matmul(
                prz[:, gi * 14 + k * 2: gi * 14 + (k + 1) * 2],
                cb["vwhhT"][:, (k * 3 + gi) * 128:(k * 3 + gi + 1) * 128],
                hs, start=False, stop=True, skip_group_check=True)
        nc.tensor.matmul(
            pn[:, k * 2:(k + 1) * 2],
            cb["vwhhT"][:, (k * 3 + 2) * 128:(k * 3 + 3) * 128],
            hs, start=False, stop=True, skip_group_check=True)
    nc.scalar.activation(
        rz[:].rearrange("p (g c) -> p g c", g=2)[:, :, lo:hi],
        prz[:, 0:28].rearrange("p (g c) -> p g c", g=2)[:, :, lo:hi],
        AF.Sigmoid)
    nc.vector.tensor_tensor(u[:, lo:hi], rz[:, lo:hi], pn[:, lo:hi],
                            op=OP.mult)
    nc.vector.tensor_tensor(
        u[:, lo:hi], u[:, lo:hi],
        vxg[:].rearrange("p (g k b v2) -> p g k b v2", g=3, k=7, b=BL)
        [:, 2, klo:khi, :, v],
        op=OP.add)
    nc.scalar.activation(nt[:, lo:hi], u[:, lo:hi], AF.Tanh)
    hsl = slice(lo - hofs, hi - hofs)
    nc.vector.tensor_tensor(t3[:, lo:hi], vh_prev[:, hsl], nt[:, lo:hi],
                            op=OP.subtract)
    nc.vector.tensor_tensor(t3[:, lo:hi], t3[:, lo:hi],
                            rz[:, 14 + lo:14 + hi], op=OP.mult)
    nc.vector.tensor_tensor(vh_new[:, hsl], t3[:, lo:hi], nt[:, lo:hi],
                            op=OP.add)


# --------------------------------------------------------------------------
# entry point
# --------------------------------------------------------------------------

_NC_CACHE = None


def kernel(**inputs):
    global _NC_CACHE
    if _NC_CACHE is None:
        _NC_CACHE = build_nc()
    nc = _NC_CACHE
    shared = _prep_shared(inputs)
    in_maps = [_prep_core(inputs, shared, c) for c in range(NCORES)]
    res = run_bass_kernel_spmd(nc, in_maps, core_ids=list(range(NCORES)))
    return np.concatenate([res.results[c]["logits"] for c in range(NCORES)],
                          axis=0).astype(np.float32)


if __name__ == "__main__":
    import reference
    inputs = {k: np.asarray(v) for k, v in reference.setup_inputs().items()}
    out = kernel(**inputs)
    print("out", out.shape, out.dtype)


# revision 49
# speedup vs baseline: 1.0467x; 1.0467x over previous
"""Trainium2 Bass kernel for nn_CrossMed4 (CrossMed-style GRU-over-GRU model).

v3: fully gather-free monitor lookups (one-hot PE matmuls), all-bf16.

The SWDGE dma_gather ucode is a hard serial floor (~8.4ns/row, one queue),
so monitor embedding lookups avoid it entirely:
- Value tables (vocab<=200, 2 vocab tiles) live in SBUF; per 128-token rank
  a one-hot OH[vocab_row, tok] is built on DVE (is_equal of host-broadcast
  ids vs an iota column) and accumulating PE matmuls OH.T @ table produce
  the value rows [tok, d] in PSUM.
- Item tables are binned ON HOST: each chunk's 3072 tokens are sorted by
  128-row vocab window, packed into ranks with a fixed rank->window plan, so
  each rank needs exactly ONE one-hot matmul. Bin overflow is evicted to one
  "spare" rank per chunk whose item rows come from a single up-front SWDGE
  gather per stream.
- Binning permutes tokens, so the sum-over-codes uses a host-streamed 0/1
  matrix S [tok, group] per rank (PE matmul, accumulated over ranks into one
  PSUM group tile).
- Both GRU levels in transposed layout (H^T [D=128, batch*keys]); bf16
  weights (fast-weight-load); steps split into two key-groups to pipeline
  PE/ACT/DVE across the serial recurrence. The weight/age visit-GRU keys
  (5,6) run before the monitor phase since they don't depend on it.
"""
import numpy as np
import ml_dtypes

try:
    import concourse.bass as bass  # noqa: F401
except ImportError:
    import sys
    sys.path.insert(0, "/opt/trn_rl_repo")

import concourse.bacc as bacc
import concourse.bass as bass
import concourse.mybir as mybir
import concourse.tile as tile
from concourse.bass_utils import run_bass_kernel_spmd

F32 = mybir.dt.float32
BF16 = mybir.dt.bfloat16
FP16 = mybir.dt.float16
F8 = mybir.dt.float8e4
I16 = mybir.dt.int16
BF = ml_dtypes.bfloat16
F8NP = ml_dtypes.float8_e4m3
AF = mybir.ActivationFunctionType
OP = mybir.AluOpType

B, V, M, L, D, OUT = 16, 16, 32, 24, 128, 193
VOCAB = {"cond": 5000, "proc": 2000, "drug": 600, "lab_item": 700,
         "lab_value": 200, "inj_item": 400, "inj_value": 200}
NCORES = 8
BL = B // NCORES            # 2 patients per core
NBV = BL * V                # 32 visit groups
TCH = 4                     # monitor steps per chunk
NCHUNK = M // TCH           # 8
GC = NBV * TCH              # 128 groups per chunk
NTOK = GC * L               # 3072 real tokens per chunk per stream
VRANKS = (NBV + 4) // 5     # 7
VIDX = VRANKS * 128         # 896

# rank -> item vocab window plan (binned ranks; one extra spare rank each)
RPLAN = {"lab": [0] * 5 + [1] * 5 + [2] * 5 + [3] * 5 + [4] * 5 + [5] * 3,
         "inj": [0] * 8 + [1] * 8 + [2] * 8 + [3] * 2}
NWIN = {"lab": 6, "inj": 4}
NBIN = {s: len(RPLAN[s]) for s in ("lab", "inj")}      # 28 / 26
NRANK = {s: NBIN[s] + 1 for s in ("lab", "inj")}       # incl spare: 29 / 27
CSW = {s: NRANK[s] * 128 for s in ("lab", "inj")}      # chunk-stream width
ITEM_OF = {"lab": "lab_item", "inj": "inj_item"}
VAL_OF = {"lab": "lab_value", "inj": "inj_value"}
# chunks whose VALUE rows come via SWDGE gather (on the otherwise-idle
# GpSimd) instead of one-hot matmuls, relieving PE/ACT/DVE
CONV = ((6, "lab"), (6, "inj"), (7, "lab"), (7, "inj"))
# iota const layout: cols 0-1 value tiles, 2..7 lab windows, 8..11 inj windows
IOTA_I0 = {"lab": 2, "inj": 8}

WSHAPES = (("mwhhT", [128, 1920]), ("mwihT34", [128, 768]),
           ("mxgb34", [1, 768]), ("mwihT012", [128, 1152]),
           ("mxgb012", [1, 1152]), ("vwhhT", [128, 2688]),
           ("vwihT04", [128, 1920]), ("vxgb04", [1, 1920]),
           ("vxg56u", [1, 768]), ("vxg56c", [1, 768]),
           ("S5", [128, 5]), ("ones", [1, 224]),
           ("wa", [1, 64]), ("fcw", [128, 7 * OUT]), ("fcb", [1, OUT]),
           ("vtab_lab", [128, 256]), ("vtab_inj", [128, 256]),
           ("itab_lab", [128, NWIN["lab"] * 128]),
           ("itab_inj", [128, NWIN["inj"] * 128]))
WSHAPES_F32 = (("bhn_bc", [128, 160]), ("vbhn_bc", [128, 14]),
               ("iotas", [128, 12]))


# --------------------------------------------------------------------------
# host-side index / weight packing
# --------------------------------------------------------------------------

def _wrap_idx(flat):
    # token i lives at [i % 16, i // 16]; the gather ucode's Q7 cores each
    # read their own 16-partition band, so replicate to all 8 bands.
    n = flat.shape[0]
    return np.tile(flat.reshape(n // 16, 16).T, (8, 1)).astype(np.int16)


def _build_visit_idx(tok):
    flat = np.zeros(VIDX, dtype=np.int64)
    t = np.asarray(tok)
    for r in range(VRANKS):
        for j in range(5):
            slot = 5 * r + j
            if slot >= NBV:
                continue
            b, v = divmod(slot, V)
            flat[r * 128 + j * 24: r * 128 + j * 24 + 24] = t[b, v, :]
    return _wrap_idx(flat)


def _bin_chunk(ids_i, ids_v, grp, sname):
    """Bin one chunk-stream's tokens by item vocab window.

    ids_i/ids_v/grp: [NTOK] arrays (item id, value id, group id 0..127).
    Returns (ibc_row, vbc_row, S, spare_idx) where rows are [CSW] and
    S is [128, CSW] one-hot token->group.
    """
    nrank, nbin = NRANK[sname], NBIN[sname]
    W = CSW[sname]
    win = ids_i >> 7
    order = np.lexsort((grp, win))
    ids_i, ids_v, grp, win = (a[order] for a in (ids_i, ids_v, grp, win))
    plan = np.asarray(RPLAN[sname])
    ib = np.full(W, -1, dtype=np.int64)  # ids reduced by window base
    vb = np.zeros(W, dtype=np.int64)
    gr = np.full(W, -1, dtype=np.int64)
    spare = []
    pos = 0
    for w in range(NWIN[sname]):
        ranks = np.nonzero(plan == w)[0]
        cap = len(ranks) * 128
        base = ranks[0] * 128
        nw = int(np.searchsorted(win, w + 1)) - pos
        take = min(nw, cap)
        sl = slice(pos, pos + take)
        ib[base:base + take] = ids_i[sl] - 128 * w
        vb[base:base + take] = ids_v[sl]
        gr[base:base + take] = grp[sl]
        if nw > take:  # bin overflow -> spare rank
            spare.extend(range(pos + take, pos + nw))
        pos += nw
    assert len(spare) <= 128, f"spare overflow: {len(spare)}"
    sb = nbin * 128
    sp_idx = np.zeros(128, dtype=np.int64)
    if spare:
        sp = np.asarray(spare)
        n = len(sp)
        sp_idx[:n] = ids_i[sp]
        vb[sb:sb + n] = ids_v[sp]
        gr[sb:sb + n] = grp[sp]
    S = np.zeros((128, W), dtype=np.float32)
    cols = np.arange(W)
    real = gr >= 0
    S[cols[real] % 128, (cols[real] // 128) * 128 + gr[real]] = 1.0
    # S layout fix: S[p, r*128 + g]; p is the slot-within-rank
    return ib, vb, S, sp_idx


def _prep_shared(inputs):
    """Weight repacking shared by all cores (pure layout transforms)."""
    f = {k: np.asarray(v, dtype=np.float32) for k, v in inputs.items()
         if not k.startswith("tok_")}
    sh = {}
    mwih, mwhh = f["mgru_wih"], f["mgru_whh"]
    mbih, mbhh = f["mgru_bih"], f["mgru_bhh"]
    vwih, vwhh = f["vgru_wih"], f["vgru_whh"]
    vbih, vbhh = f["vgru_bih"], f["vgru_bhh"]

    def packT(w_keys):  # [K, 3D, D] -> [128, K*3*128], col (k*3+gi)*128+gu
        k = w_keys.shape[0]
        out = np.zeros((128, k * 3 * 128), dtype=np.float32)
        for ki in range(k):
            for gi in range(3):
                out[:, (ki * 3 + gi) * 128:(ki * 3 + gi + 1) * 128] = \
                    w_keys[ki, gi * 128:(gi + 1) * 128, :].T
        return out

    def pack_xgb(bih, bhh, keys):  # -> [1, len(keys)*384]
        rows = []
        for k in keys:
            b = bih[k].copy()
            b[:2 * D] += bhh[k][:2 * D]
            rows.append(b)
        return np.concatenate(rows)[None, :]

    sh["mwhhT"] = packT(mwhh)
    sh["mwihT34"] = packT(mwih[3:5])
    sh["mxgb34"] = pack_xgb(mbih, mbhh, [3, 4])
    sh["mwihT012"] = packT(mwih[0:3])
    sh["mxgb012"] = pack_xgb(mbih, mbhh, [0, 1, 2])
    sh["bhn_bc"] = np.repeat(mbhh[:, 2 * D:].T, NBV, axis=1).astype(np.float32)
    sh["vwhhT"] = packT(vwhh)
    sh["vwihT04"] = packT(vwih[0:5])
    sh["vxgb04"] = pack_xgb(vbih, vbhh, [0, 1, 2, 3, 4])
    u_rows, c_rows = [], []
    for k in (5, 6):
        u_rows.append(vwih[k] @ f["info_w"][k - 5])
        cv = vwih[k] @ f["info_b"][k - 5] + vbih[k]
        cv[:2 * D] += vbhh[k][:2 * D]
        c_rows.append(cv)
    sh["vxg56u"] = np.concatenate(u_rows)[None, :]
    sh["vxg56c"] = np.concatenate(c_rows)[None, :]
    sh["vbhn_bc"] = np.repeat(vbhh[:, 2 * D:].T, BL, axis=1).astype(np.float32)
    s5 = np.zeros((128, 5), dtype=np.float32)
    for j in range(5):
        s5[j * 24:(j + 1) * 24, j] = 1.0
    sh["S5"] = s5
    sh["ones"] = np.ones((1, 224), dtype=np.float32)
    fcw = np.zeros((128, 7 * OUT), dtype=np.float32)
    for k in range(7):
        fcw[:, k * OUT:(k + 1) * OUT] = f["fc_w"][k * D:(k + 1) * D, :]
    sh["fcw"] = fcw
    sh["fcb"] = f["fc_b"][None, :]
    for s in ("lab", "inj"):
        tabf = f["emb_" + VAL_OF[s]]
        vt = np.zeros((128, 256), dtype=np.float32)
        vt[:, 0:128] = tabf[0:128]
        vt[:tabf.shape[0] - 128, 128:256] = tabf[128:]
        sh["vtab_" + s] = vt
        tabi = f["emb_" + ITEM_OF[s]]
        it = np.zeros((128, NWIN[s] * 128), dtype=np.float32)
        for w in range(NWIN[s]):
            rows = tabi[w * 128:(w + 1) * 128]
            it[:rows.shape[0], w * 128:w * 128 + 128] = rows
        sh["itab_" + s] = it
    iv = np.zeros((128, 12), dtype=np.float32)
    p = np.arange(128)
    iv[:, 0] = p
    iv[:, 1] = p + 128
    for s in ("lab", "inj"):
        for w in range(NWIN[s]):
            iv[:, IOTA_I0[s] + w] = p + 128 * w
    sh["iotas"] = iv
    out = {}
    for n, _ in WSHAPES:
        if n == "wa":
            continue
        out[n] = sh[n].astype(BF)
    for n, _ in WSHAPES_F32:
        out[n] = sh[n].astype(np.float32)
    for name in ("cond", "proc", "drug", "lab_item", "inj_item",
                 "lab_value", "inj_value"):
        out["emb_" + name] = f["emb_" + name].astype(BF)
    return out


def _prep_core(inputs, shared, core):
    b0 = core * BL
    m = dict(shared)
    for name in ("cond", "proc", "drug"):
        m["idx_" + name] = _build_visit_idx(
            np.asarray(inputs["tok_" + name])[b0:b0 + BL])
    for sname in ("lab", "inj"):
        ti = np.asarray(inputs["tok_" + ITEM_OF[sname]])[b0:b0 + BL] \
            .reshape(NBV, M, L)
        tv = np.asarray(inputs["tok_" + VAL_OF[sname]])[b0:b0 + BL] \
            .reshape(NBV, M, L)
        sp_flat = np.zeros(NCHUNK * 128, dtype=np.int64)
        for c in range(NCHUNK):
            msl = slice(c * TCH, (c + 1) * TCH)
            ids_i = ti[:, msl, :].transpose(1, 0, 2).reshape(-1)
            ids_v = tv[:, msl, :].transpose(1, 0, 2).reshape(-1)
            grp = (np.arange(TCH)[:, None, None] * NBV +
                   np.arange(NBV)[None, :, None] +
                   0 * np.arange(L)[None, None, :]).reshape(-1)
            ib, vb, S, sp = _bin_chunk(ids_i, ids_v, grp, sname)
            m[f"ibc_{sname}_{c}"] = np.ascontiguousarray(
                np.broadcast_to(ib.astype(BF)[None, :], (128, CSW[sname])))
            if (c, sname) in CONV:
                m[f"vidx_{sname}_{c}"] = _wrap_idx(vb)
            else:
                m[f"vbc_{sname}_{c}"] = np.ascontiguousarray(
                    np.broadcast_to(vb.astype(BF)[None, :], (128, CSW[sname])))
            m[f"smat_{sname}_{c}"] = S.astype(F8NP)
            sp_flat[c * 128:(c + 1) * 128] = sp
        m[f"spidx_{sname}"] = _wrap_idx(sp_flat)
    wa = np.zeros((1, 64), dtype=np.float32)
    wa[0, :NBV] = np.asarray(inputs["weight"], np.float32)[b0:b0 + BL].reshape(NBV)
    wa[0, NBV:] = np.asarray(inputs["age"], np.float32)[b0:b0 + BL].reshape(NBV)
    m["wa"] = wa.astype(BF)
    return m


# --------------------------------------------------------------------------
# device program
# --------------------------------------------------------------------------

def build_nc():
    nc = bacc.Bacc("TRN2", target_bir_lowering=False, debug=False,
                   num_devices=NCORES)
    dt = {}
    for name in ("cond", "proc", "drug", "lab_item", "inj_item",
                 "lab_value", "inj_value"):
        dt["emb_" + name] = nc.dram_tensor("emb_" + name, [VOCAB[name], D],
                                           BF16, kind="ExternalInput")
    for name in ("cond", "proc", "drug"):
        dt["idx_" + name] = nc.dram_tensor("idx_" + name, [128, VIDX // 16],
                                           I16, kind="ExternalInput")
    for s in ("lab", "inj"):
        dt[f"spidx_{s}"] = nc.dram_tensor(f"spidx_{s}", [128, NCHUNK * 8],
                                          I16, kind="ExternalInput")
        for c in range(NCHUNK):
            dt[f"ibc_{s}_{c}"] = nc.dram_tensor(f"ibc_{s}_{c}",
                                                [128, CSW[s]], BF16,
                                                kind="ExternalInput")
            if (c, s) in CONV:
                dt[f"vidx_{s}_{c}"] = nc.dram_tensor(
                    f"vidx_{s}_{c}", [128, CSW[s] // 16], I16,
                    kind="ExternalInput")
            else:
                dt[f"vbc_{s}_{c}"] = nc.dram_tensor(f"vbc_{s}_{c}",
                                                    [128, CSW[s]], BF16,
                                                    kind="ExternalInput")
            dt[f"smat_{s}_{c}"] = nc.dram_tensor(f"smat_{s}_{c}",
                                                 [128, CSW[s]], F8,
                                                 kind="ExternalInput")
    for name, shape in WSHAPES:
        dt[name] = nc.dram_tensor(name, shape, BF16, kind="ExternalInput")
    for name, shape in WSHAPES_F32:
        dt[name] = nc.dram_tensor(name, shape, F32, kind="ExternalInput")
    out_logits = nc.dram_tensor("logits", [BL, OUT], F32, kind="ExternalOutput")

    with tile.TileContext(nc) as tc:
        _program(nc, tc, dt, out_logits)
    nc.compile()
    return nc


def _program(nc, tc, dt, out_logits):
    import contextlib
    ctx = contextlib.ExitStack()
    with ctx:
        cpool = ctx.enter_context(tc.tile_pool(name="const", bufs=1))
        gpool = ctx.enter_context(tc.tile_pool(name="gather", bufs=1))
        vbpool = ctx.enter_context(tc.tile_pool(name="vb", bufs=2))
        ohpool = ctx.enter_context(tc.tile_pool(name="oh", bufs=2))
        prpool = ctx.enter_context(tc.tile_pool(name="pr", bufs=2))
        spool = ctx.enter_context(tc.tile_pool(name="work", bufs=2))
        xgpool = ctx.enter_context(tc.tile_pool(name="xg34", bufs=3))
        hpool = ctx.enter_context(tc.tile_pool(name="h", bufs=2))
        p2 = ctx.enter_context(tc.tile_pool(name="psum2", bufs=2,
                                            space="PSUM"))
        p1 = ctx.enter_context(tc.tile_pool(name="psum1", bufs=1,
                                            space="PSUM"))

        # ---- load constants to SBUF; critical-path ones first, then chunk-0
        # stream data, then the rest (the sync queue issues in order).
        shapes = dict(WSHAPES)
        shapes_f32 = dict(WSHAPES_F32)
        cb = {}

        def _load(name):
            if name in shapes_f32:
                t = cpool.tile(shapes_f32[name], F32, tag=name)
            else:
                t = cpool.tile(shapes[name], BF16, tag=name)
            nc.sync.dma_start(t[:], dt[name].ap())
            cb[name] = t

        idx = {}
        for name in ("cond", "proc", "drug"):
            t = cpool.tile([128, VIDX // 16], I16, tag="idx_" + name)
            nc.sync.dma_start(t[:], dt["idx_" + name].ap())
            idx[name] = t
        for name in ("iotas", "vtab_lab", "vtab_inj", "itab_lab", "itab_inj",
                     "S5", "ones", "mwihT012", "mxgb012", "mwhhT", "bhn_bc",
                     "mwihT34", "mxgb34", "vxg56u", "vxg56c", "wa", "vwhhT",
                     "vbhn_bc"):
            _load(name)

        # chunk-stream tile prefetch (vb/ib/st for chunk c)
        stream_tiles = {}

        def _prefetch(c):
            if c >= NCHUNK:
                return
            for sname in ("lab", "inj"):
                W = CSW[sname]
                if (c, sname) in CONV:
                    vb = None
                else:
                    vb = vbpool.tile([128, W], BF16, tag="vb_" + sname)
                    nc.sync.dma_start(vb[:], dt[f"vbc_{sname}_{c}"].ap())
                ib = vbpool.tile([128, W], BF16, tag="ib_" + sname)
                nc.sync.dma_start(ib[:], dt[f"ibc_{sname}_{c}"].ap())
                st = vbpool.tile([128, W], F8, tag="st_" + sname)
                nc.sync.dma_start(st[:], dt[f"smat_{sname}_{c}"].ap())
                stream_tiles[(c, sname)] = (vb, ib, st)

        _prefetch(0)
        spi = {}
        for s in ("lab", "inj"):
            t = cpool.tile([128, NCHUNK * 8], I16, tag="spidx_" + s)
            nc.sync.dma_start(t[:], dt[f"spidx_{s}"].ap())
            spi[s] = t
        for name in ("vwihT04", "vxgb04", "fcw", "fcb"):
            _load(name)

        S5 = cb["S5"]
        ones = cb["ones"]
        iotas = cb["iotas"]

        # ---- visit-level features eT[k] = [128, 32] bf16
        eT = {}
        for name in ("cond", "proc", "drug"):
            gt = gpool.tile([128, VIDX], BF16, tag="vgather")
            nc.gpsimd.dma_gather(
                gt[:].rearrange("p (r e) -> p r e", e=D),
                dt["emb_" + name].ap(), idx[name][:], VIDX, VIDX, D)
            pr = p1.tile([128, 128], F32, tag="red")
            for r in range(VRANKS):
                nc.tensor.matmul(pr[:, 5 * r:5 * r + 5],
                                 gt[:, r * D:(r + 1) * D], S5[:],
                                 start=True, stop=True)
            et = cpool.tile([128, NBV], BF16, tag="eT_" + name)
            nc.scalar.copy(et[:], pr[:, :NBV])
            eT[name] = et

        # ---- spare-rank item gathers (one per stream, all chunks)
        spg = {}
        for s in ("lab", "inj"):
            gt = cpool.tile([128, NCHUNK * 128], BF16, tag="spg_" + s)
            nc.gpsimd.dma_gather(
                gt[:].rearrange("p (r e) -> p r e", e=D),
                dt["emb_" + ITEM_OF[s]].ap(), spi[s][:],
                NCHUNK * 128, NCHUNK * 128, D)
            spg[s] = gt

        # ---- value-row gathers for the CONV chunks run just-in-time on the
        # otherwise-idle GpSimd, two chunks ahead of use
        convt = {}
        vix = {}
        for cc, s in CONV:
            t = cpool.tile([128, CSW[s] // 16], I16, tag=f"vidx_{s}_{cc}")
            nc.sync.dma_start(t[:], dt[f"vidx_{s}_{cc}"].ap())
            vix[(cc, s)] = t

        def _conv_gather(cc):
            for s in ("lab", "inj"):
                if (cc, s) not in CONV:
                    continue
                W = CSW[s]
                vt = vbpool.tile([128, W], BF16, tag="vt_" + s)
                for r0 in range(0, NRANK[s], 8):
                    nr = min(8, NRANK[s] - r0)
                    nc.gpsimd.dma_gather(
                        vt[:].rearrange("p (r e) -> p r e", e=D)
                        [:, r0:r0 + nr, :],
                        dt["emb_" + VAL_OF[s]].ap(),
                        vix[(cc, s)][:, r0 * 8:(r0 + nr) * 8],
                        nr * 128, nr * 128, D)
                convt[(cc, s)] = vt

        # ---- XGc for monitor keys 0-2: [128, 3*96], col gi*96 + k*32 + bv
        xgc = cpool.tile([128, 288], BF16, tag="xgc")
        for k, name in enumerate(("cond", "proc", "drug")):
            pk = p1.tile([128, 384], F32, tag="xg")
            for gi in range(3):
                off = (k * 3 + gi) * 128
                nc.tensor.matmul(pk[:, gi * 32:(gi + 1) * 32],
                                 cb["mwihT012"][:, off:off + 128],
                                 eT[name][:], start=True, stop=False)
                nc.tensor.matmul(pk[:, gi * 32:(gi + 1) * 32],
                                 cb["mxgb012"][0:1, off:off + 128],
                                 ones[0:1, 0:NBV], start=False, stop=True)
            nc.scalar.copy(
                xgc[:].rearrange("p (g c) -> p g c", g=3)[:, :, k * 32:(k + 1) * 32],
                pk[:, 0:96].rearrange("p (g c) -> p g c", g=3))

        # ---- visit GRU xg for keys 5,6 (weight/age)
        vxg = cpool.tile([128, 672], BF16, tag="vxg")  # col gi*224 + k*32 + bv
        for k in (5, 6):
            pk = p1.tile([128, 384], F32, tag="xg")
            for gi in range(3):
                off = ((k - 5) * 3 + gi) * 128
                nc.tensor.matmul(pk[:, gi * 32:(gi + 1) * 32],
                                 cb["vxg56u"][0:1, off:off + 128],
                                 cb["wa"][0:1, (k - 5) * 32:(k - 4) * 32],
                                 start=True, stop=False)
                nc.tensor.matmul(pk[:, gi * 32:(gi + 1) * 32],
                                 cb["vxg56c"][0:1, off:off + 128],
                                 ones[0:1, 0:NBV], start=False, stop=True)
            nc.scalar.copy(
                vxg[:].rearrange("p (g c) -> p g c", g=3)[:, :, k * 32:(k + 1) * 32],
                pk[:, 0:96].rearrange("p (g c) -> p g c", g=3))

        # ---- visit GRU keys 5,6 final state (steps interleaved into chunks)
        vh56 = cpool.tile([128, 4], BF16, tag="vh56")

        # ---- monitor chunks + GRU
        h_prev = hpool.tile([128, 160], BF16, tag="H")
        nc.vector.memset(h_prev[:], 0.0)

        vh0 = hpool.tile([128, 4], BF16, tag="VH56")
        nc.vector.memset(vh0[:], 0.0)
        vh56_state = [vh0]

        def emit_vh56(v):
            # one keys-5/6 visit-GRU step, hidden in chunk slack
            pz = p2.tile([128, 42], F32, tag="v56")
            rz = spool.tile([128, 28], BF16, tag="vrz")
            u = spool.tile([128, 14], BF16, tag="vu")
            nt = spool.tile([128, 14], BF16, tag="vnt")
            t3 = spool.tile([128, 14], BF16, tag="vt3")
            vh_new = vh56 if v == V - 1 else hpool.tile([128, 4], BF16,
                                                        tag="VH56")
            _vgru_sub(nc, cb, vxg, pz[:, 0:28], pz[:, 28:42], rz, u, nt, t3,
                      vh56_state[0], vh_new, v, 5, 7, hofs=10)
            vh56_state[0] = vh_new

        for c in range(NCHUNK):
            xg34 = xgpool.tile([128, 768], BF16, tag="xg34")
            _prefetch(c + 1)
            if c + 4 in [cc for cc, _ in CONV]:
                _conv_gather(c + 4)
            emit_vh56(2 * c)
            for k, sname in ((3, "lab"), (4, "inj")):
                W = CSW[sname]
                nbin, nrank = NBIN[sname], NRANK[sname]
                vb, ib, st = stream_tiles.pop((c, sname))
                conv = (c, sname) in CONV
                vtab, itab = cb["vtab_" + sname], cb["itab_" + sname]
                ohi = ohpool.tile([128, W], BF16, tag="ohi")
                if not conv:
                    oh0 = ohpool.tile([128, W], BF16, tag="oh0")
                    oh1 = ohpool.tile([128, W], BF16, tag="oh1")
                    nc.vector.tensor_scalar(oh0[:], vb[:], iotas[:, 0:1],
                                            None, OP.is_equal)
                    nc.vector.tensor_scalar(oh1[:], vb[:], iotas[:, 1:2],
                                            None, OP.is_equal)
                plan = RPLAN[sname]
                nc.vector.tensor_scalar(
                    ohi[:, 0:nbin * 128], ib[:, 0:nbin * 128],
                    iotas[:, 0:1], None, OP.is_equal)
                xsum = p1.tile([128, 128], F32, tag="red")
                s_pending = []
                for r0 in range(0, nrank, 4):
                    nr = min(4, nrank - r0)
                    wd = nr * 128
                    if conv:
                        vtile, vofs = convt[(c, sname)], r0 * 128
                    else:
                        pv = p2.tile([128, 512], F32, tag="pv")
                        for j in range(nr):
                            sl = slice((r0 + j) * 128, (r0 + j + 1) * 128)
                            nc.tensor.matmul(pv[:, j * 128:(j + 1) * 128],
                                             oh0[:, sl], vtab[:, 0:128],
                                             start=True, stop=False)
                            nc.tensor.matmul(pv[:, j * 128:(j + 1) * 128],
                                             oh1[:, sl], vtab[:, 128:256],
                                             start=False, stop=True)
                        vtile = prpool.tile([128, 512], BF16, tag="pvb")
                        nc.scalar.copy(vtile[:, 0:wd], pv[:, 0:wd])
                        vofs = 0
                    pt = prpool.tile([128, 512], BF16, tag="prod")
                    nbin_j = min(nr, nbin - r0)  # binned ranks in this group
                    if nbin_j > 0:
                        pi = p2.tile([128, 512], F32, tag="pv")
                        for j in range(nbin_j):
                            w = plan[r0 + j]
                            sl = slice((r0 + j) * 128, (r0 + j + 1) * 128)
                            nc.tensor.matmul(
                                pi[:, j * 128:(j + 1) * 128],
                                ohi[:, sl], itab[:, w * 128:(w + 1) * 128],
                                start=True, stop=True)
                        nc.vector.tensor_tensor(pt[:, 0:nbin_j * 128],
                                                pi[:, 0:nbin_j * 128],
                                                vtile[:, vofs:vofs + nbin_j * 128],
                                                op=OP.mult)
                    if nbin_j < nr:  # spare rank (always last)
                        j = nbin_j
                        nc.vector.tensor_tensor(
                            pt[:, j * 128:(j + 1) * 128],
                            spg[sname][:, c * 128:(c + 1) * 128],
                            vtile[:, vofs + j * 128:vofs + (j + 1) * 128],
                            op=OP.mult)
                    for args in s_pending:
                        nc.tensor.matmul(*args[:3], start=args[3], stop=args[4],
                                         skip_group_check=True)
                    s_pending = []
                    for j in range(nr):
                        r = r0 + j
                        s_pending.append(
                            (xsum[:], pt[:, j * 128:(j + 1) * 128],
                             st[:, r * 128:(r + 1) * 128],
                             r == 0, r == nrank - 1))
                for args in s_pending:
                    nc.tensor.matmul(*args[:3], start=args[3], stop=args[4],
                                     skip_group_check=True)
                xmT = spool.tile([128, GC], BF16, tag="xmT")
                nc.scalar.copy(xmT[:], xsum[:])
                pxg = p1.tile([128, 384], F32, tag="xg")
                for gi in range(3):
                    off = ((k - 3) * 3 + gi) * 128
                    nc.tensor.matmul(pxg[:, gi * 128:(gi + 1) * 128],
                                     cb["mwihT34"][:, off:off + 128],
                                     xmT[:], start=True, stop=False)
                    nc.tensor.matmul(pxg[:, gi * 128:(gi + 1) * 128],
                                     cb["mxgb34"][0:1, off:off + 128],
                                     ones[0:1, 0:128], start=False, stop=True)
                # copy into xg34: col gi*256 + mi*64 + (k-3)*32 + bv
                for gi in range(3):
                    nc.scalar.copy(
                        xg34[:].rearrange("p (g m kb) -> p g m kb", g=3, m=TCH)
                        [:, gi, :, (k - 3) * 32:(k - 2) * 32],
                        pxg[:].rearrange("p (g m b) -> p g m b", g=3, m=TCH)
                        [:, gi, :, :])
                if sname == "lab":
                    emit_vh56(2 * c + 1)

            for mi in range(TCH):
                gt_ = p2.tile([128, 480], F32, tag="gru")
                prz = gt_[:, 0:320]
                pn = gt_[:, 320:480]
                rz = spool.tile([128, 320], BF16, tag="rz")
                npre = spool.tile([128, 160], BF16, tag="npre")
                nt = spool.tile([128, 160], BF16, tag="nt")
                t3 = spool.tile([128, 160], BF16, tag="t3")
                h_new = hpool.tile([128, 160], BF16, tag="H")
                # two key-groups: A = keys 0-2 (cols 0:96), B = keys 3-4
                for klo, khi in ((0, 3), (3, 5)):
                    lo, hi = klo * 32, khi * 32
                    if klo == 0:
                        nc.scalar.copy(
                            prz.rearrange("p (g c) -> p g c", g=2)[:, :, 0:96],
                            xgc[:].rearrange("p (g c) -> p g c", g=3)[:, 0:2, :])
                    else:
                        nc.scalar.copy(
                            prz.rearrange("p (g c) -> p g c", g=2)[:, :, 96:160],
                            xg34[:].rearrange("p (g c) -> p g c", g=3)
                            [:, 0:2, mi * 64:(mi + 1) * 64])
                    nc.scalar.copy(pn[:, lo:hi], cb["bhn_bc"][:, lo:hi])
                    for k in range(klo, khi):
                        hs = h_prev[:, k * 32:(k + 1) * 32]
                        for gi in range(2):
                            nc.tensor.matmul(
                                prz[:, gi * 160 + k * 32: gi * 160 + (k + 1) * 32],
                                cb["mwhhT"][:, (k * 3 + gi) * 128:(k * 3 + gi + 1) * 128],
                                hs, start=False, stop=True, skip_group_check=True)
                        nc.tensor.matmul(
                            pn[:, k * 32:(k + 1) * 32],
                            cb["mwhhT"][:, (k * 3 + 2) * 128:(k * 3 + 3) * 128],
                            hs, start=False, stop=True, skip_group_check=True)
                    nc.scalar.activation(
                        rz[:].rearrange("p (g c) -> p g c", g=2)[:, :, lo:hi],
                        prz.rearrange("p (g c) -> p g c", g=2)[:, :, lo:hi],
                        AF.Sigmoid)
                    nc.vector.tensor_tensor(npre[:, lo:hi], rz[:, lo:hi],
                                            pn[:, lo:hi], op=OP.mult)
                    if klo == 0:
                        nc.vector.tensor_tensor(
                            npre[:, lo:hi], npre[:, lo:hi],
                            xgc[:].rearrange("p (g c) -> p g c", g=3)[:, 2, :],
                            op=OP.add)
                    else:
                        nc.vector.tensor_tensor(
                            npre[:, lo:hi], npre[:, lo:hi],
                            xg34[:, 2 * 256 + mi * 64: 2 * 256 + (mi + 1) * 64],
                            op=OP.add)
                    nc.scalar.activation(nt[:, lo:hi], npre[:, lo:hi], AF.Tanh)
                    nc.vector.tensor_tensor(t3[:, lo:hi], h_prev[:, lo:hi],
                                            nt[:, lo:hi], op=OP.subtract)
                    nc.vector.tensor_tensor(t3[:, lo:hi], t3[:, lo:hi],
                                            rz[:, 160 + lo:160 + hi],
                                            op=OP.mult)
                    nc.vector.tensor_tensor(h_new[:, lo:hi], t3[:, lo:hi],
                                            nt[:, lo:hi], op=OP.add)
                h_prev = h_new



        # ---- visit GRU xg for keys 0-4 (from final monitor h)
        for k in range(5):
            pk = p1.tile([128, 384], F32, tag="xg")
            for gi in range(3):
                off = (k * 3 + gi) * 128
                nc.tensor.matmul(pk[:, gi * 32:(gi + 1) * 32],
                                 cb["vwihT04"][:, off:off + 128],
                                 h_prev[:, k * 32:(k + 1) * 32],
                                 start=True, stop=False)
                nc.tensor.matmul(pk[:, gi * 32:(gi + 1) * 32],
                                 cb["vxgb04"][0:1, off:off + 128],
                                 ones[0:1, 0:NBV], start=False, stop=True)
            nc.scalar.copy(
                vxg[:].rearrange("p (g c) -> p g c", g=3)[:, :, k * 32:(k + 1) * 32],
                pk[:, 0:96].rearrange("p (g c) -> p g c", g=3))

        # ---- visit GRU keys 0-4 (16 steps)
        vh_prev = hpool.tile([128, 10], BF16, tag="VH")
        nc.vector.memset(vh_prev[:], 0.0)
        for v in range(V):
            gt_ = p2.tile([128, 480], F32, tag="gru")
            rz = spool.tile([128, 28], BF16, tag="vrz")
            u = spool.tile([128, 14], BF16, tag="vu")
            nt = spool.tile([128, 14], BF16, tag="vnt")
            t3 = spool.tile([128, 14], BF16, tag="vt3")
            vh_new = hpool.tile([128, 10], BF16, tag="VH")
            for klo, khi in ((0, 2), (2, 5)):
                _vgru_sub(nc, cb, vxg, gt_[:, 0:320], gt_[:, 320:480],
                          rz, u, nt, t3, vh_prev, vh_new, v, klo, khi, hofs=0)
            vh_prev = vh_new

        # ---- FC head
        rl = spool.tile([128, 14], BF16, tag="rl")
        nc.scalar.activation(rl[:, 0:10], vh_prev[:], AF.Relu)
        nc.scalar.activation(rl[:, 10:14], vh56[:], AF.Relu)
        pfc = p1.tile([BL, 384], F32, tag="xg")
        for k in range(7):
            nc.tensor.matmul(pfc[:, 0:OUT], rl[:, k * 2:(k + 1) * 2],
                             cb["fcw"][:, k * OUT:(k + 1) * OUT],
                             start=(k == 0), stop=False)
        nc.tensor.matmul(pfc[:, 0:OUT], ones[0:1, 0:BL], cb["fcb"][0:1, :],
                         start=False, stop=True)
        lg = spool.tile([BL, OUT], F32, tag="lg")
        nc.scalar.copy(lg[:], pfc[:, 0:OUT])
        nc.sync.dma_start(out_logits.ap(), lg[:])


def _vgru_sub(nc, cb, vxg, prz, pn, rz, u, nt, t3, vh_prev, vh_new,
              v, klo, khi, hofs):
    """One key-subrange of one visit-GRU step.

    vh_prev/vh_new are [128, 2*(khi-klo...)] slices indexed relative to hofs:
    h column for key k lives at (k*2 - hofs*2//...) -- callers pass tiles
    whose col 0 corresponds to key hofs//2... simplified: vh tiles hold keys
    [hofs/2, ...) with col (k - hofs_key)*2.
    """
    kofs = hofs // 2  # first key held in the vh tiles
    lo, hi = klo * 2, khi * 2
    nc.vector.tensor_scalar(
        prz[:, 0:28].rearrange("p (g c) -> p g c", g=2)[:, :, lo:hi],
        vxg[:].rearrange("p (g k b v2) -> p g k b v2", g=3, k=7, b=BL)
        [:, 0:2, klo:khi, :, v], 0.0, None, OP.add)
    nc.vector.tensor_scalar(pn[:, lo:hi], cb["vbhn_bc"][:, lo:hi],
                            0.0, None, OP.add)
    for k in range(klo, khi):
        hs = vh_prev[:, (k - kofs) * 2:(k - kofs + 1) * 2]
        for gi in range(2):
            nc.tensor.matmul(
                prz[:, gi * 14 + k * 2: gi * 14 + (k + 1) * 2],
                cb["vwhhT"][:, (k * 3 + gi) * 128:(k * 3 + gi + 1) * 128],
                hs, start=False, stop=True, skip_group_check=True)
        nc.tensor.matmul(
            pn[:, k * 2:(k + 1) * 2],
            cb["vwhhT"][:, (k * 3 + 2) * 128:(k * 3 + 3) * 128],
            hs, start=False, stop=True, skip_group_check=True)
    nc.scalar.activation(
        rz[:].rearrange("p (g c) -> p g c", g=2)[:, :, lo:hi],
        prz[:, 0:28].rearrange("p (g c) -> p g c", g=2)[:, :, lo:hi],
        AF.Sigmoid)
    nc.vector.tensor_tensor(u[:, lo:hi], rz[:, lo:hi], pn[:, lo:hi],
                            op=OP.mult)
    nc.vector.tensor_tensor(
        u[:, lo:hi], u[:, lo:hi],
        vxg[:].rearrange("p (g k b v2) -> p g k b v2", g=3, k=7, b=BL)
        [:, 2, klo:khi, :, v],
        op=OP.add)
    nc.scalar.activation(nt[:, lo:hi], u[:, lo:hi], AF.Tanh)
    hsl = slice(lo - hofs, hi - hofs)
    nc.vector.tensor_tensor(t3[:, lo:hi], vh_prev[:, hsl], nt[:, lo:hi],
                            op=OP.subtract)
    nc.vector.tensor_tensor(t3[:, lo:hi], t3[:, lo:hi],
                            rz[:, 14 + lo:14 + hi], op=OP.mult)
    nc.vector.tensor_tensor(vh_new[:, hsl], t3[:, lo:hi], nt[:, lo:hi],
                            op=OP.add)


# --------------------------------------------------------------------------
# entry point
# --------------------------------------------------------------------------

_NC_CACHE = None


def kernel(**inputs):
    global _NC_CACHE
    if _NC_CACHE is None:
        _NC_CACHE = build_nc()
    nc = _NC_CACHE
    shared = _prep_shared(inputs)
    in_maps = [_prep_core(inputs, shared, c) for c in range(NCORES)]
    res = run_bass_kernel_spmd(nc, in_maps, core_ids=list(range(NCORES)))
    return np.concatenate([res.results[c]["logits"] for c in range(NCORES)],
                          axis=0).astype(np.float32)


if __name__ == "__main__":
    import reference
    inputs = {k: np.asarray(v) for k, v in reference.setup_inputs().items()}
    out = kernel(**inputs)
    print("out", out.shape, out.dtype)


# revision 50
# speedup vs baseline: 1.0931x; 1.0443x over previous
"""Trainium2 Bass kernel for nn_CrossMed4 (CrossMed-style GRU-over-GRU model).

v3: fully gather-free monitor lookups (one-hot PE matmuls), all-bf16.

The SWDGE dma_gather ucode is a hard serial floor (~8.4ns/row, one queue),
so monitor embedding lookups avoid it entirely:
- Value tables (vocab<=200, 2 vocab tiles) live in SBUF; per 128-token rank
  a one-hot OH[vocab_row, tok] is built on DVE (is_equal of host-broadcast
  ids vs an iota column) and accumulating PE matmuls OH.T @ table produce
  the value rows [tok, d] in PSUM.
- Item tables are binned ON HOST: each chunk's 3072 tokens are sorted by
  128-row vocab window, packed into ranks with a fixed rank->window plan, so
  each rank needs exactly ONE one-hot matmul. Bin overflow is evicted to one
  "spare" rank per chunk whose item rows come from a single up-front SWDGE
  gather per stream.
- Binning permutes tokens, so the sum-over-codes uses a host-streamed 0/1
  matrix S [tok, group] per rank (PE matmul, accumulated over ranks into one
  PSUM group tile).
- Both GRU levels in transposed layout (H^T [D=128, batch*keys]); bf16
  weights (fast-weight-load); steps split into two key-groups to pipeline
  PE/ACT/DVE across the serial recurrence. The weight/age visit-GRU keys
  (5,6) run before the monitor phase since they don't depend on it.
"""
import numpy as np
import ml_dtypes

try:
    import concourse.bass as bass  # noqa: F401
except ImportError:
    import sys
    sys.path.insert(0, "/opt/trn_rl_repo")

import concourse.bacc as bacc
import concourse.bass as bass
import concourse.mybir as mybir
import concourse.tile as tile
from concourse.bass_utils import run_bass_kernel_spmd

F32 = mybir.dt.float32
BF16 = mybir.dt.bfloat16
FP16 = mybir.dt.float16
F8 = mybir.dt.float8e4
I16 = mybir.dt.int16
BF = ml_dtypes.bfloat16
F8NP = ml_dtypes.float8_e4m3
AF = mybir.ActivationFunctionType
OP = mybir.AluOpType

B, V, M, L, D, OUT = 16, 16, 32, 24, 128, 193
VOCAB = {"cond": 5000, "proc": 2000, "drug": 600, "lab_item": 700,
         "lab_value": 200, "inj_item": 400, "inj_value": 200}
NCORES = 8
BL = B // NCORES            # 2 patients per core
NBV = BL * V                # 32 visit groups
TCH = 4                     # monitor steps per chunk
NCHUNK = M // TCH           # 8
GC = NBV * TCH              # 128 groups per chunk
NTOK = GC * L               # 3072 real tokens per chunk per stream
VRANKS = (NBV + 4) // 5     # 7
VIDX = VRANKS * 128         # 896

# rank -> item vocab window plan (binned ranks; one extra spare rank each)
RPLAN = {"lab": [0] * 5 + [1] * 5 + [2] * 5 + [3] * 5 + [4] * 5 + [5] * 3,
         "inj": [0] * 8 + [1] * 8 + [2] * 8 + [3] * 2}
NWIN = {"lab": 6, "inj": 4}
NBIN = {s: len(RPLAN[s]) for s in ("lab", "inj")}      # 28 / 26
NRANK = {s: NBIN[s] + 1 for s in ("lab", "inj")}       # incl spare: 29 / 27
CSW = {s: NRANK[s] * 128 for s in ("lab", "inj")}      # chunk-stream width
ITEM_OF = {"lab": "lab_item", "inj": "inj_item"}
VAL_OF = {"lab": "lab_value", "inj": "inj_value"}
# chunks whose VALUE rows come via SWDGE gather (on the otherwise-idle
# GpSimd) instead of one-hot matmuls, relieving PE/ACT/DVE
CONV = ((6, "lab"), (6, "inj"), (7, "lab"), (7, "inj"))
# iota const layout: cols 0-1 value tiles, 2..7 lab windows, 8..11 inj windows
IOTA_I0 = {"lab": 2, "inj": 8}

WSHAPES = (("mwhhT", [128, 1920]), ("mwihT34", [128, 768]),
           ("mxgb34", [1, 768]), ("mwihT012", [128, 1152]),
           ("mxgb012", [1, 1152]), ("vwhhT", [128, 2688]),
           ("vwihT04", [128, 1920]), ("vxgb04", [1, 1920]),
           ("vxg56u", [1, 768]), ("vxg56c", [1, 768]),
           ("S5", [128, 5]), ("ones", [1, 224]),
           ("wa", [1, 64]), ("fcw", [128, 7 * OUT]), ("fcb", [1, OUT]),
           ("vtab_lab", [128, 256]), ("vtab_inj", [128, 256]),
           ("itab_lab", [128, NWIN["lab"] * 128]),
           ("itab_inj", [128, NWIN["inj"] * 128]))
WSHAPES_F32 = (("bhn_bc", [128, 160]), ("vbhn_bc", [128, 14]),
               ("iotas", [128, 12]))


# --------------------------------------------------------------------------
# host-side index / weight packing
# --------------------------------------------------------------------------

def _wrap_idx(flat):
    # token i lives at [i % 16, i // 16]; the gather ucode's Q7 cores each
    # read their own 16-partition band, so replicate to all 8 bands.
    n = flat.shape[0]
    return np.tile(flat.reshape(n // 16, 16).T, (8, 1)).astype(np.int16)


def _build_visit_idx(tok):
    flat = np.zeros(VIDX, dtype=np.int64)
    t = np.asarray(tok)
    for r in range(VRANKS):
        for j in range(5):
            slot = 5 * r + j
            if slot >= NBV:
                continue
            b, v = divmod(slot, V)
            flat[r * 128 + j * 24: r * 128 + j * 24 + 24] = t[b, v, :]
    return _wrap_idx(flat)


def _bin_chunk(ids_i, ids_v, grp, sname):
    """Bin one chunk-stream's tokens by item vocab window.

    ids_i/ids_v/grp: [NTOK] arrays (item id, value id, group id 0..127).
    Returns (ibc_row, vbc_row, S, spare_idx) where rows are [CSW] and
    S is [128, CSW] one-hot token->group.
    """
    nrank, nbin = NRANK[sname], NBIN[sname]
    W = CSW[sname]
    win = ids_i >> 7
    order = np.lexsort((grp, win))
    ids_i, ids_v, grp, win = (a[order] for a in (ids_i, ids_v, grp, win))
    plan = np.asarray(RPLAN[sname])
    ib = np.full(W, -1, dtype=np.int64)  # ids reduced by window base
    vb = np.zeros(W, dtype=np.int64)
    gr = np.full(W, -1, dtype=np.int64)
    spare = []
    pos = 0
    for w in range(NWIN[sname]):
        ranks = np.nonzero(plan == w)[0]
        cap = len(ranks) * 128
        base = ranks[0] * 128
        nw = int(np.searchsorted(win, w + 1)) - pos
        take = min(nw, cap)
        sl = slice(pos, pos + take)
        ib[base:base + take] = ids_i[sl] - 128 * w
        vb[base:base + take] = ids_v[sl]
        gr[base:base + take] = grp[sl]
        if nw > take:  # bin overflow -> spare rank
            spare.extend(range(pos + take, pos + nw))
        pos += nw
    assert len(spare) <= 128, f"spare overflow: {len(spare)}"
    sb = nbin * 128
    sp_idx = np.zeros(128, dtype=np.int64)
    if spare:
        sp = np.asarray(spare)
        n = len(sp)
        sp_idx[:n] = ids_i[sp]
        vb[sb:sb + n] = ids_v[sp]
        gr[sb:sb + n] = grp[sp]
    S = np.zeros((128, W), dtype=np.float32)
    cols = np.arange(W)
    real = gr >= 0
    S[cols[real] % 128, (cols[real] // 128) * 128 + gr[real]] = 1.0
    # S layout fix: S[p, r*128 + g]; p is the slot-within-rank
    return ib, vb, S, sp_idx


def _prep_shared(inputs):
    """Weight repacking shared by all cores (pure layout transforms)."""
    f = {k: np.asarray(v, dtype=np.float32) for k, v in inputs.items()
         if not k.startswith("tok_")}
    sh = {}
    mwih, mwhh = f["mgru_wih"], f["mgru_whh"]
    mbih, mbhh = f["mgru_bih"], f["mgru_bhh"]
    vwih, vwhh = f["vgru_wih"], f["vgru_whh"]
    vbih, vbhh = f["vgru_bih"], f["vgru_bhh"]

    def packT(w_keys):  # [K, 3D, D] -> [128, K*3*128], col (k*3+gi)*128+gu
        k = w_keys.shape[0]
        out = np.zeros((128, k * 3 * 128), dtype=np.float32)
        for ki in range(k):
            for gi in range(3):
                out[:, (ki * 3 + gi) * 128:(ki * 3 + gi + 1) * 128] = \
                    w_keys[ki, gi * 128:(gi + 1) * 128, :].T
        return out

    def pack_xgb(bih, bhh, keys):  # -> [1, len(keys)*384]
        rows = []
        for k in keys:
            b = bih[k].copy()
            b[:2 * D] += bhh[k][:2 * D]
            rows.append(b)
        return np.concatenate(rows)[None, :]

    sh["mwhhT"] = packT(mwhh)
    sh["mwihT34"] = packT(mwih[3:5])
    sh["mxgb34"] = pack_xgb(mbih, mbhh, [3, 4])
    sh["mwihT012"] = packT(mwih[0:3])
    sh["mxgb012"] = pack_xgb(mbih, mbhh, [0, 1, 2])
    sh["bhn_bc"] = np.repeat(mbhh[:, 2 * D:].T, NBV, axis=1).astype(np.float32)
    sh["vwhhT"] = packT(vwhh)
    sh["vwihT04"] = packT(vwih[0:5])
    sh["vxgb04"] = pack_xgb(vbih, vbhh, [0, 1, 2, 3, 4])
    u_rows, c_rows = [], []
    for k in (5, 6):
        u_rows.append(vwih[k] @ f["info_w"][k - 5])
        cv = vwih[k] @ f["info_b"][k - 5] + vbih[k]
        cv[:2 * D] += vbhh[k][:2 * D]
        c_rows.append(cv)
    sh["vxg56u"] = np.concatenate(u_rows)[None, :]
    sh["vxg56c"] = np.concatenate(c_rows)[None, :]
    sh["vbhn_bc"] = np.repeat(vbhh[:, 2 * D:].T, BL, axis=1).astype(np.float32)
    s5 = np.zeros((128, 5), dtype=np.float32)
    for j in range(5):
        s5[j * 24:(j + 1) * 24, j] = 1.0
    sh["S5"] = s5
    sh["ones"] = np.ones((1, 224), dtype=np.float32)
    fcw = np.zeros((128, 7 * OUT), dtype=np.float32)
    for k in range(7):
        fcw[:, k * OUT:(k + 1) * OUT] = f["fc_w"][k * D:(k + 1) * D, :]
    sh["fcw"] = fcw
    sh["fcb"] = f["fc_b"][None, :]
    for s in ("lab", "inj"):
        tabf = f["emb_" + VAL_OF[s]]
        vt = np.zeros((128, 256), dtype=np.float32)
        vt[:, 0:128] = tabf[0:128]
        vt[:tabf.shape[0] - 128, 128:256] = tabf[128:]
        sh["vtab_" + s] = vt
        tabi = f["emb_" + ITEM_OF[s]]
        it = np.zeros((128, NWIN[s] * 128), dtype=np.float32)
        for w in range(NWIN[s]):
            rows = tabi[w * 128:(w + 1) * 128]
            it[:rows.shape[0], w * 128:w * 128 + 128] = rows
        sh["itab_" + s] = it
    iv = np.zeros((128, 12), dtype=np.float32)
    p = np.arange(128)
    iv[:, 0] = p
    iv[:, 1] = p + 128
    for s in ("lab", "inj"):
        for w in range(NWIN[s]):
            iv[:, IOTA_I0[s] + w] = p + 128 * w
    sh["iotas"] = iv
    out = {}
    for n, _ in WSHAPES:
        if n == "wa":
            continue
        out[n] = sh[n].astype(BF)
    for n, _ in WSHAPES_F32:
        out[n] = sh[n].astype(np.float32)
    for name in ("cond", "proc", "drug", "lab_item", "inj_item",
                 "lab_value", "inj_value"):
        out["emb_" + name] = f["emb_" + name].astype(BF)
    return out


def _prep_core(inputs, shared, core):
    b0 = core * BL
    m = dict(shared)
    for name in ("cond", "proc", "drug"):
        m["idx_" + name] = _build_visit_idx(
            np.asarray(inputs["tok_" + name])[b0:b0 + BL])
    for sname in ("lab", "inj"):
        ti = np.asarray(inputs["tok_" + ITEM_OF[sname]])[b0:b0 + BL] \
            .reshape(NBV, M, L)
        tv = np.asarray(inputs["tok_" + VAL_OF[sname]])[b0:b0 + BL] \
            .reshape(NBV, M, L)
        sp_flat = np.zeros(NCHUNK * 128, dtype=np.int64)
        for c in range(NCHUNK):
            msl = slice(c * TCH, (c + 1) * TCH)
            ids_i = ti[:, msl, :].transpose(1, 0, 2).reshape(-1)
            ids_v = tv[:, msl, :].transpose(1, 0, 2).reshape(-1)
            grp = (np.arange(TCH)[:, None, None] * NBV +
                   np.arange(NBV)[None, :, None] +
                   0 * np.arange(L)[None, None, :]).reshape(-1)
            ib, vb, S, sp = _bin_chunk(ids_i, ids_v, grp, sname)
            m[f"ibc_{sname}_{c}"] = np.ascontiguousarray(
                np.broadcast_to(ib.astype(BF)[None, :], (128, CSW[sname])))
            if (c, sname) in CONV:
                m[f"vidx_{sname}_{c}"] = _wrap_idx(vb)
            else:
                m[f"vbc_{sname}_{c}"] = np.ascontiguousarray(
                    np.broadcast_to(vb.astype(BF)[None, :], (128, CSW[sname])))
            m[f"smat_{sname}_{c}"] = S.astype(F8NP)
            sp_flat[c * 128:(c + 1) * 128] = sp
        m[f"spidx_{sname}"] = _wrap_idx(sp_flat)
    wa = np.zeros((1, 64), dtype=np.float32)
    wa[0, :NBV] = np.asarray(inputs["weight"], np.float32)[b0:b0 + BL].reshape(NBV)
    wa[0, NBV:] = np.asarray(inputs["age"], np.float32)[b0:b0 + BL].reshape(NBV)
    m["wa"] = wa.astype(BF)
    return m


# --------------------------------------------------------------------------
# device program
# --------------------------------------------------------------------------

def build_nc():
    nc = bacc.Bacc("TRN2", target_bir_lowering=False, debug=False,
                   num_devices=NCORES)
    dt = {}
    for name in ("cond", "proc", "drug", "lab_item", "inj_item",
                 "lab_value", "inj_value"):
        dt["emb_" + name] = nc.dram_tensor("emb_" + name, [VOCAB[name], D],
                                           BF16, kind="ExternalInput")
    for name in ("cond", "proc", "drug"):
        dt["idx_" + name] = nc.dram_tensor("idx_" + name, [128, VIDX // 16],
                                           I16, kind="ExternalInput")
    for s in ("lab", "inj"):
        dt[f"spidx_{s}"] = nc.dram_tensor(f"spidx_{s}", [128, NCHUNK * 8],
                                          I16, kind="ExternalInput")
        for c in range(NCHUNK):
            dt[f"ibc_{s}_{c}"] = nc.dram_tensor(f"ibc_{s}_{c}",
                                                [128, CSW[s]], BF16,
                                                kind="ExternalInput")
            if (c, s) in CONV:
                dt[f"vidx_{s}_{c}"] = nc.dram_tensor(
                    f"vidx_{s}_{c}", [128, CSW[s] // 16], I16,
                    kind="ExternalInput")
            else:
                dt[f"vbc_{s}_{c}"] = nc.dram_tensor(f"vbc_{s}_{c}",
                                                    [128, CSW[s]], BF16,
                                                    kind="ExternalInput")
            dt[f"smat_{s}_{c}"] = nc.dram_tensor(f"smat_{s}_{c}",
                                                 [128, CSW[s]], F8,
                                                 kind="ExternalInput")
    for name, shape in WSHAPES:
        dt[name] = nc.dram_tensor(name, shape, BF16, kind="ExternalInput")
    for name, shape in WSHAPES_F32:
        dt[name] = nc.dram_tensor(name, shape, F32, kind="ExternalInput")
    out_logits = nc.dram_tensor("logits", [BL, OUT], F32, kind="ExternalOutput")

    with tile.TileContext(nc) as tc:
        _program(nc, tc, dt, out_logits)
    nc.compile()
    return nc


def _program(nc, tc, dt, out_logits):
    import contextlib
    ctx = contextlib.ExitStack()
    with ctx:
        cpool = ctx.enter_context(tc.tile_pool(name="const", bufs=1))
        gpool = ctx.enter_context(tc.tile_pool(name="gather", bufs=2))
        vbpool = ctx.enter_context(tc.tile_pool(name="vb", bufs=2))
        ohpool = ctx.enter_context(tc.tile_pool(name="oh", bufs=2))
        prpool = ctx.enter_context(tc.tile_pool(name="pr", bufs=4))
        spool = ctx.enter_context(tc.tile_pool(name="work", bufs=2))
        xgpool = ctx.enter_context(tc.tile_pool(name="xg34", bufs=3))
        hpool = ctx.enter_context(tc.tile_pool(name="h", bufs=2))
        p2 = ctx.enter_context(tc.tile_pool(name="psum2", bufs=2,
                                            space="PSUM"))
        p1 = ctx.enter_context(tc.tile_pool(name="psum1", bufs=1,
                                            space="PSUM"))

        # ---- load constants to SBUF; critical-path ones first, then chunk-0
        # stream data, then the rest (the sync queue issues in order).
        shapes = dict(WSHAPES)
        shapes_f32 = dict(WSHAPES_F32)
        cb = {}

        def _load(name):
            if name in shapes_f32:
                t = cpool.tile(shapes_f32[name], F32, tag=name)
            else:
                t = cpool.tile(shapes[name], BF16, tag=name)
            nc.sync.dma_start(t[:], dt[name].ap())
            cb[name] = t

        idx = {}
        for name in ("cond", "proc", "drug"):
            t = cpool.tile([128, VIDX // 16], I16, tag="idx_" + name)
            nc.sync.dma_start(t[:], dt["idx_" + name].ap())
            idx[name] = t
        for name in ("iotas", "vtab_lab", "vtab_inj", "itab_lab", "itab_inj",
                     "S5", "ones", "mwihT012", "mxgb012", "mwhhT", "bhn_bc",
                     "mwihT34", "mxgb34", "vxg56u", "vxg56c", "wa", "vwhhT",
                     "vbhn_bc"):
            _load(name)

        # chunk-stream tile prefetch (vb/ib/st for chunk c)
        stream_tiles = {}

        def _prefetch(c):
            if c >= NCHUNK:
                return
            for sname in ("lab", "inj"):
                W = CSW[sname]
                if (c, sname) in CONV:
                    vb = None
                else:
                    vb = vbpool.tile([128, W], BF16, tag="vb_" + sname)
                    nc.sync.dma_start(vb[:], dt[f"vbc_{sname}_{c}"].ap())
                ib = vbpool.tile([128, W], BF16, tag="ib_" + sname)
                nc.sync.dma_start(ib[:], dt[f"ibc_{sname}_{c}"].ap())
                st = vbpool.tile([128, W], F8, tag="st_" + sname)
                nc.sync.dma_start(st[:], dt[f"smat_{sname}_{c}"].ap())
                stream_tiles[(c, sname)] = (vb, ib, st)

        _prefetch(0)
        spi = {}
        for s in ("lab", "inj"):
            t = cpool.tile([128, NCHUNK * 8], I16, tag="spidx_" + s)
            nc.sync.dma_start(t[:], dt[f"spidx_{s}"].ap())
            spi[s] = t
        for name in ("vwihT04", "vxgb04", "fcw", "fcb"):
            _load(name)

        S5 = cb["S5"]
        ones = cb["ones"]
        iotas = cb["iotas"]

        # ---- visit-level features eT[k] = [128, 32] bf16
        eT = {}
        for name in ("cond", "proc", "drug"):
            gt = gpool.tile([128, VIDX], BF16, tag="vgather")
            nc.gpsimd.dma_gather(
                gt[:].rearrange("p (r e) -> p r e", e=D),
                dt["emb_" + name].ap(), idx[name][:], VIDX, VIDX, D)
            pr = p1.tile([128, 128], F32, tag="red")
            for r in range(VRANKS):
                nc.tensor.matmul(pr[:, 5 * r:5 * r + 5],
                                 gt[:, r * D:(r + 1) * D], S5[:],
                                 start=True, stop=True)
            et = cpool.tile([128, NBV], BF16, tag="eT_" + name)
            nc.scalar.copy(et[:], pr[:, :NBV])
            eT[name] = et

        # ---- spare-rank item gathers (one per stream, all chunks)
        spg = {}
        for s in ("lab", "inj"):
            gt = cpool.tile([128, NCHUNK * 128], BF16, tag="spg_" + s)
            nc.gpsimd.dma_gather(
                gt[:].rearrange("p (r e) -> p r e", e=D),
                dt["emb_" + ITEM_OF[s]].ap(), spi[s][:],
                NCHUNK * 128, NCHUNK * 128, D)
            spg[s] = gt

        # ---- value-row gathers for the CONV chunks run just-in-time on the
        # otherwise-idle GpSimd, two chunks ahead of use
        convt = {}
        vix = {}
        for cc, s in CONV:
            t = cpool.tile([128, CSW[s] // 16], I16, tag=f"vidx_{s}_{cc}")
            nc.sync.dma_start(t[:], dt[f"vidx_{s}_{cc}"].ap())
            vix[(cc, s)] = t

        def _conv_gather(cc):
            for s in ("lab", "inj"):
                if (cc, s) not in CONV:
                    continue
                W = CSW[s]
                vt = vbpool.tile([128, W], BF16, tag="vt_" + s)
                for r0 in range(0, NRANK[s], 8):
                    nr = min(8, NRANK[s] - r0)
                    nc.gpsimd.dma_gather(
                        vt[:].rearrange("p (r e) -> p r e", e=D)
                        [:, r0:r0 + nr, :],
                        dt["emb_" + VAL_OF[s]].ap(),
                        vix[(cc, s)][:, r0 * 8:(r0 + nr) * 8],
                        nr * 128, nr * 128, D)
                convt[(cc, s)] = vt

        # ---- XGc for monitor keys 0-2: [128, 3*96], col gi*96 + k*32 + bv
        xgc = cpool.tile([128, 288], BF16, tag="xgc")
        for k, name in enumerate(("cond", "proc", "drug")):
            pk = p1.tile([128, 384], F32, tag="xg")
            for gi in range(3):
                off = (k * 3 + gi) * 128
                nc.tensor.matmul(pk[:, gi * 32:(gi + 1) * 32],
                                 cb["mwihT012"][:, off:off + 128],
                                 eT[name][:], start=True, stop=False)
                nc.tensor.matmul(pk[:, gi * 32:(gi + 1) * 32],
                                 cb["mxgb012"][0:1, off:off + 128],
                                 ones[0:1, 0:NBV], start=False, stop=True)
            nc.scalar.copy(
                xgc[:].rearrange("p (g c) -> p g c", g=3)[:, :, k * 32:(k + 1) * 32],
                pk[:, 0:96].rearrange("p (g c) -> p g c", g=3))

        # ---- visit GRU xg for keys 5,6 (weight/age)
        vxg = cpool.tile([128, 672], BF16, tag="vxg")  # col gi*224 + k*32 + bv
        for k in (5, 6):
            pk = p1.tile([128, 384], F32, tag="xg")
            for gi in range(3):
                off = ((k - 5) * 3 + gi) * 128
                nc.tensor.matmul(pk[:, gi * 32:(gi + 1) * 32],
                                 cb["vxg56u"][0:1, off:off + 128],
                                 cb["wa"][0:1, (k - 5) * 32:(k - 4) * 32],
                                 start=True, stop=False)
                nc.tensor.matmul(pk[:, gi * 32:(gi + 1) * 32],
                                 cb["vxg56c"][0:1, off:off + 128],
                                 ones[0:1, 0:NBV], start=False, stop=True)
            nc.scalar.copy(
                vxg[:].rearrange("p (g c) -> p g c", g=3)[:, :, k * 32:(k + 1) * 32],
                pk[:, 0:96].rearrange("p (g c) -> p g c", g=3))

        # ---- visit GRU keys 5,6 final state (steps interleaved into chunks)
        vh56 = cpool.tile([128, 4], BF16, tag="vh56")

        # ---- monitor chunks + GRU
        h_prev = hpool.tile([128, 160], BF16, tag="H")
        nc.vector.memset(h_prev[:], 0.0)

        vh0 = hpool.tile([128, 4], BF16, tag="VH56")
        nc.vector.memset(vh0[:], 0.0)
        vh56_state = [vh0]

        def emit_vh56(v):
            # one keys-5/6 visit-GRU step, hidden in chunk slack
            pz = p2.tile([128, 42], F32, tag="v56")
            rz = spool.tile([128, 28], BF16, tag="vrz")
            u = spool.tile([128, 14], BF16, tag="vu")
            nt = spool.tile([128, 14], BF16, tag="vnt")
            t3 = spool.tile([128, 14], BF16, tag="vt3")
            vh_new = vh56 if v == V - 1 else hpool.tile([128, 4], BF16,
                                                        tag="VH56")
            _vgru_sub(nc, cb, vxg, pz[:, 0:28], pz[:, 28:42], rz, u, nt, t3,
                      vh56_state[0], vh_new, v, 5, 7, hofs=10)
            vh56_state[0] = vh_new

        for c in range(NCHUNK):
            xg34 = xgpool.tile([128, 768], BF16, tag="xg34")
            _prefetch(c + 1)
            if c + 4 in [cc for cc, _ in CONV]:
                _conv_gather(c + 4)
            emit_vh56(2 * c)
            for k, sname in ((3, "lab"), (4, "inj")):
                W = CSW[sname]
                nbin, nrank = NBIN[sname], NRANK[sname]
                vb, ib, st = stream_tiles.pop((c, sname))
                conv = (c, sname) in CONV
                vtab, itab = cb["vtab_" + sname], cb["itab_" + sname]
                ohi = ohpool.tile([128, W], BF16, tag="ohi")
                if not conv:
                    oh0 = ohpool.tile([128, W], BF16, tag="oh0")
                    oh1 = ohpool.tile([128, W], BF16, tag="oh1")
                    nc.vector.tensor_scalar(oh0[:], vb[:], iotas[:, 0:1],
                                            None, OP.is_equal)
                    nc.vector.tensor_scalar(oh1[:], vb[:], iotas[:, 1:2],
                                            None, OP.is_equal)
                plan = RPLAN[sname]
                nc.vector.tensor_scalar(
                    ohi[:, 0:nbin * 128], ib[:, 0:nbin * 128],
                    iotas[:, 0:1], None, OP.is_equal)
                xsum = p1.tile([128, 128], F32, tag="red")
                s_pending = []
                for r0 in range(0, nrank, 4):
                    nr = min(4, nrank - r0)
                    wd = nr * 128
                    if conv:
                        vtile, vofs = convt[(c, sname)], r0 * 128
                    else:
                        pv = p2.tile([128, 512], F32, tag="pv")
                        for j in range(nr):
                            sl = slice((r0 + j) * 128, (r0 + j + 1) * 128)
                            nc.tensor.matmul(pv[:, j * 128:(j + 1) * 128],
                                             oh0[:, sl], vtab[:, 0:128],
                                             start=True, stop=False)
                            nc.tensor.matmul(pv[:, j * 128:(j + 1) * 128],
                                             oh1[:, sl], vtab[:, 128:256],
                                             start=False, stop=True)
                        vtile = prpool.tile([128, 512], BF16, tag="pvb")
                        nc.scalar.copy(vtile[:, 0:wd], pv[:, 0:wd])
                        vofs = 0
                    pt = prpool.tile([128, 512], BF16, tag="prod")
                    nbin_j = min(nr, nbin - r0)  # binned ranks in this group
                    if nbin_j > 0:
                        pi = p2.tile([128, 512], F32, tag="pv")
                        for j in range(nbin_j):
                            w = plan[r0 + j]
                            sl = slice((r0 + j) * 128, (r0 + j + 1) * 128)
                            nc.tensor.matmul(
                                pi[:, j * 128:(j + 1) * 128],
                                ohi[:, sl], itab[:, w * 128:(w + 1) * 128],
                                start=True, stop=True)
                        nc.vector.tensor_tensor(pt[:, 0:nbin_j * 128],
                                                pi[:, 0:nbin_j * 128],
                                                vtile[:, vofs:vofs + nbin_j * 128],
                                                op=OP.mult)
                    if nbin_j < nr:  # spare rank (always last)
                        j = nbin_j
                        nc.vector.tensor_tensor(
                            pt[:, j * 128:(j + 1) * 128],
                            spg[sname][:, c * 128:(c + 1) * 128],
                            vtile[:, vofs + j * 128:vofs + (j + 1) * 128],
                            op=OP.mult)
                    for args in s_pending:
                        nc.tensor.matmul(*args[:3], start=args[3], stop=args[4],
                                         skip_group_check=True)
                    s_pending = []
                    for j in range(nr):
                        r = r0 + j
                        s_pending.append(
                            (xsum[:], pt[:, j * 128:(j + 1) * 128],
                             st[:, r * 128:(r + 1) * 128],
                             r == 0, r == nrank - 1))
                for args in s_pending:
                    nc.tensor.matmul(*args[:3], start=args[3], stop=args[4],
                                     skip_group_check=True)
                xmT = spool.tile([128, GC], BF16, tag="xmT")
                nc.scalar.copy(xmT[:], xsum[:])
                pxg = p1.tile([128, 384], F32, tag="xg")
                for gi in range(3):
                    off = ((k - 3) * 3 + gi) * 128
                    nc.tensor.matmul(pxg[:, gi * 128:(gi + 1) * 128],
                                     cb["mwihT34"][:, off:off + 128],
                                     xmT[:], start=True, stop=False)
                    nc.tensor.matmul(pxg[:, gi * 128:(gi + 1) * 128],
                                     cb["mxgb34"][0:1, off:off + 128],
                                     ones[0:1, 0:128], start=False, stop=True)
                # copy into xg34: col gi*256 + mi*64 + (k-3)*32 + bv
                for gi in range(3):
                    nc.scalar.copy(
                        xg34[:].rearrange("p (g m kb) -> p g m kb", g=3, m=TCH)
                        [:, gi, :, (k - 3) * 32:(k - 2) * 32],
                        pxg[:].rearrange("p (g m b) -> p g m b", g=3, m=TCH)
                        [:, gi, :, :])
                if sname == "lab":
                    emit_vh56(2 * c + 1)

            for mi in range(TCH):
                gt_ = p2.tile([128, 480], F32, tag="gru")
                prz = gt_[:, 0:320]
                pn = gt_[:, 320:480]
                rz = spool.tile([128, 320], BF16, tag="rz")
                npre = spool.tile([128, 160], BF16, tag="npre")
                nt = spool.tile([128, 160], BF16, tag="nt")
                t3 = spool.tile([128, 160], BF16, tag="t3")
                h_new = hpool.tile([128, 160], BF16, tag="H")
                # two key-groups: A = keys 0-2 (cols 0:96), B = keys 3-4
                for klo, khi in ((0, 3), (3, 5)):
                    lo, hi = klo * 32, khi * 32
                    if klo == 0:
                        nc.scalar.copy(
                            prz.rearrange("p (g c) -> p g c", g=2)[:, :, 0:96],
                            xgc[:].rearrange("p (g c) -> p g c", g=3)[:, 0:2, :])
                    else:
                        nc.scalar.copy(
                            prz.rearrange("p (g c) -> p g c", g=2)[:, :, 96:160],
                            xg34[:].rearrange("p (g c) -> p g c", g=3)
                            [:, 0:2, mi * 64:(mi + 1) * 64])
                    nc.scalar.copy(pn[:, lo:hi], cb["bhn_bc"][:, lo:hi])
                    for k in range(klo, khi):
                        hs = h_prev[:, k * 32:(k + 1) * 32]
                        for gi in range(2):
                            nc.tensor.matmul(
                                prz[:, gi * 160 + k * 32: gi * 160 + (k + 1) * 32],
                                cb["mwhhT"][:, (k * 3 + gi) * 128:(k * 3 + gi + 1) * 128],
                                hs, start=False, stop=True, skip_group_check=True)
                        nc.tensor.matmul(
                            pn[:, k * 32:(k + 1) * 32],
                            cb["mwhhT"][:, (k * 3 + 2) * 128:(k * 3 + 3) * 128],
                            hs, start=False, stop=True, skip_group_check=True)
                    nc.scalar.activation(
                        rz[:].rearrange("p (g c) -> p g c", g=2)[:, :, lo:hi],
                        prz.rearrange("p (g c) -> p g c", g=2)[:, :, lo:hi],
                        AF.Sigmoid)
                    nc.vector.tensor_tensor(npre[:, lo:hi], rz[:, lo:hi],
                                            pn[:, lo:hi], op=OP.mult)
                    if klo == 0:
                        nc.vector.tensor_tensor(
                            npre[:, lo:hi], npre[:, lo:hi],
                            xgc[:].rearrange("p (g c) -> p g c", g=3)[:, 2, :],
                            op=OP.add)
                    else:
                        nc.vector.tensor_tensor(
                            npre[:, lo:hi], npre[:, lo:hi],
                            xg34[:, 2 * 256 + mi * 64: 2 * 256 + (mi + 1) * 64],
                            op=OP.add)
                    nc.scalar.activation(nt[:, lo:hi], npre[:, lo:hi], AF.Tanh)
                    nc.vector.tensor_tensor(t3[:, lo:hi], h_prev[:, lo:hi],
                                            nt[:, lo:hi], op=OP.subtract)
                    nc.vector.tensor_tensor(t3[:, lo:hi], t3[:, lo:hi],
                                            rz[:, 160 + lo:160 + hi],
                                            op=OP.mult)
                    nc.vector.tensor_tensor(h_new[:, lo:hi], t3[:, lo:hi],
                                            nt[:, lo:hi], op=OP.add)
                h_prev = h_new



        # ---- visit GRU xg for keys 0-4 (from final monitor h)
        for k in range(5):
            pk = p1.tile([128, 384], F32, tag="xg")
            for gi in range(3):
                off = (k * 3 + gi) * 128
                nc.tensor.matmul(pk[:, gi * 32:(gi + 1) * 32],
                                 cb["vwihT04"][:, off:off + 128],
                                 h_prev[:, k * 32:(k + 1) * 32],
                                 start=True, stop=False)
                nc.tensor.matmul(pk[:, gi * 32:(gi + 1) * 32],
                                 cb["vxgb04"][0:1, off:off + 128],
                                 ones[0:1, 0:NBV], start=False, stop=True)
            nc.scalar.copy(
                vxg[:].rearrange("p (g c) -> p g c", g=3)[:, :, k * 32:(k + 1) * 32],
                pk[:, 0:96].rearrange("p (g c) -> p g c", g=3))

        # ---- visit GRU keys 0-4 (16 steps)
        vh_prev = hpool.tile([128, 10], BF16, tag="VH")
        nc.vector.memset(vh_prev[:], 0.0)
        for v in range(V):
            gt_ = p2.tile([128, 480], F32, tag="gru")
            rz = spool.tile([128, 28], BF16, tag="vrz")
            u = spool.tile([128, 14], BF16, tag="vu")
            nt = spool.tile([128, 14], BF16, tag="vnt")
            t3 = spool.tile([128, 14], BF16, tag="vt3")
            vh_new = hpool.tile([128, 10], BF16, tag="VH")
            for klo, khi in ((0, 2), (2, 5)):
                _vgru_sub(nc, cb, vxg, gt_[:, 0:320], gt_[:, 320:480],
                          rz, u, nt, t3, vh_prev, vh_new, v, klo, khi, hofs=0)
            vh_prev = vh_new

        # ---- FC head
        rl = spool.tile([128, 14], BF16, tag="rl")
        nc.scalar.activation(rl[:, 0:10], vh_prev[:], AF.Relu)
        nc.scalar.activation(rl[:, 10:14], vh56[:], AF.Relu)
        pfc = p1.tile([BL, 384], F32, tag="xg")
        for k in range(7):
            nc.tensor.matmul(pfc[:, 0:OUT], rl[:, k * 2:(k + 1) * 2],
                             cb["fcw"][:, k * OUT:(k + 1) * OUT],
                             start=(k == 0), stop=False)
        nc.tensor.matmul(pfc[:, 0:OUT], ones[0:1, 0:BL], cb["fcb"][0:1, :],
                         start=False, stop=True)
        lg = spool.tile([BL, OUT], F32, tag="lg")
        nc.scalar.copy(lg[:], pfc[:, 0:OUT])
        nc.sync.dma_start(out_logits.ap(), lg[:])


def _vgru_sub(nc, cb, vxg, prz, pn, rz, u, nt, t3, vh_prev, vh_new,
              v, klo, khi, hofs):
    """One key-subrange of one visit-GRU step.

    vh_prev/vh_new are [128, 2*(khi-klo...)] slices indexed relative to hofs:
    h column for key k lives at (k*2 - hofs*2//...) -- callers pass tiles
    whose col 0 corresponds to key hofs//2... simplified: vh tiles hold keys
    [hofs/2, ...) with col (k - hofs_key)*2.
    """
    kofs = hofs // 2  # first key held in the vh tiles
    lo, hi = klo * 2, khi * 2
    nc.vector.tensor_scalar(
        prz[:, 0:28].rearrange("p (g c) -> p g c", g=2)[:, :, lo:hi],
        vxg[:].rearrange("p (g k b v2) -> p g k b v2", g=3, k=7, b=BL)
        [:, 0:2, klo:khi, :, v], 0.0, None, OP.add)
    nc.vector.tensor_scalar(pn[:, lo:hi], cb["vbhn_bc"][:, lo:hi],
                            0.0, None, OP.add)
    for k in range(klo, khi):
        hs = vh_prev[:, (k - kofs) * 2:(k - kofs + 1) * 2]
        for gi in range(2):
            nc.tensor.matmul(
                prz[:, gi * 14 + k * 2: gi * 14 + (k + 1) * 2],
                cb["vwhhT"][:, (k * 3 + gi) * 128:(k * 3 + gi + 1) * 128],
                hs, start=False, stop=True, skip_group_check=True)
        nc.tensor.matmul(
            pn[:, k * 2:(k + 1) * 2],
            cb["vwhhT"][:, (k * 3 + 2) * 128:(k * 3 + 3) * 128],
            hs, start=False, stop=True, skip_group_check=True)
    nc.scalar.activation(
        rz[:].rearrange("p (g c) -> p g c", g=2)[:, :, lo:hi],
        prz[:, 0:28].rearrange("p (g c) -> p g c", g=2)[:, :, lo:hi],
        AF.Sigmoid)
    nc.vector.tensor_tensor(u[:, lo:hi], rz[:, lo:hi], pn[:, lo:hi],
                            op=OP.mult)
    nc.vector.tensor_tensor(
        u[:, lo:hi], u[:, lo:hi],
        vxg[:].rearrange("p (g k b v2) -> p g k b v2", g=3, k=7, b=BL)
        [:, 2, klo:khi, :, v],
        op=OP.add)
    nc.scalar.activation(nt[:, lo:hi], u[:, lo:hi], AF.Tanh)
    hsl = slice(lo - hofs, hi - hofs)
    nc.vector.tensor_tensor(t3[:, lo:hi], vh_prev[:, hsl], nt[:, lo:hi],
                            op=OP.subtract)
    nc.vector.tensor_tensor(t3[:, lo:hi], t3[:, lo:hi],
                            rz[:, 14 + lo:14 + hi], op=OP.mult)
    nc.vector.tensor_tensor(vh_new[:, hsl], t3[:, lo:hi], nt[:, lo:hi],
                            op=OP.add)


# --------------------------------------------------------------------------
# entry point
# --------------------------------------------------------------------------

_NC_CACHE = None


def kernel(**inputs):
    global _NC_CACHE
    if _NC_CACHE is None:
        _NC_CACHE = build_nc()
    nc = _NC_CACHE
    shared = _prep_shared(inputs)
    in_maps = [_prep_core(inputs, shared, c) for c in range(NCORES)]
    res = run_bass_kernel_spmd(nc, in_maps, core_ids=list(range(NCORES)))
    return np.concatenate([res.results[c]["logits"] for c in range(NCORES)],
                          axis=0).astype(np.float32)


if __name__ == "__main__":
    import reference
    inputs = {k: np.asarray(v) for k, v in reference.setup_inputs().items()}
    out = kernel(**inputs)
    print("out", out.shape, out.dtype)


# revision 51
# speedup vs baseline: 1.1711x; 1.0713x over previous
"""Trainium2 Bass kernel for nn_CrossMed4 (CrossMed-style GRU-over-GRU model).

v3: fully gather-free monitor lookups (one-hot PE matmuls), all-bf16.

The SWDGE dma_gather ucode is a hard serial floor (~8.4ns/row, one queue),
so monitor embedding lookups avoid it entirely:
- Value tables (vocab<=200, 2 vocab tiles) live in SBUF; per 128-token rank
  a one-hot OH[vocab_row, tok] is built on DVE (is_equal of host-broadcast
  ids vs an iota column) and accumulating PE matmuls OH.T @ table produce
  the value rows [tok, d] in PSUM.
- Item tables are binned ON HOST: each chunk's 3072 tokens are sorted by
  128-row vocab window, packed into ranks with a fixed rank->window plan, so
  each rank needs exactly ONE one-hot matmul. Bin overflow is evicted to one
  "spare" rank per chunk whose item rows come from a single up-front SWDGE
  gather per stream.
- Binning permutes tokens, so the sum-over-codes uses a host-streamed 0/1
  matrix S [tok, group] per rank (PE matmul, accumulated over ranks into one
  PSUM group tile).
- Both GRU levels in transposed layout (H^T [D=128, batch*keys]); bf16
  weights (fast-weight-load); steps split into two key-groups to pipeline
  PE/ACT/DVE across the serial recurrence. The weight/age visit-GRU keys
  (5,6) run before the monitor phase since they don't depend on it.
"""
import numpy as np
import ml_dtypes

try:
    import concourse.bass as bass  # noqa: F401
except ImportError:
    import sys
    sys.path.insert(0, "/opt/trn_rl_repo")

import concourse.bacc as bacc
import concourse.bass as bass
import concourse.mybir as mybir
import concourse.tile as tile
from concourse.bass_utils import run_bass_kernel_spmd

F32 = mybir.dt.float32
BF16 = mybir.dt.bfloat16
FP16 = mybir.dt.float16
F8 = mybir.dt.float8e4
I16 = mybir.dt.int16
BF = ml_dtypes.bfloat16
F8NP = ml_dtypes.float8_e4m3
AF = mybir.ActivationFunctionType
OP = mybir.AluOpType

B, V, M, L, D, OUT = 16, 16, 32, 24, 128, 193
VOCAB = {"cond": 5000, "proc": 2000, "drug": 600, "lab_item": 700,
         "lab_value": 200, "inj_item": 400, "inj_value": 200}
NCORES = 8
BL = B // NCORES            # 2 patients per core
NBV = BL * V                # 32 visit groups
TCH = 4                     # monitor steps per chunk
NCHUNK = M // TCH           # 8
GC = NBV * TCH              # 128 groups per chunk
NTOK = GC * L               # 3072 real tokens per chunk per stream
VRANKS = (NBV + 4) // 5     # 7
VIDX = VRANKS * 128         # 896

# rank -> item vocab window plan (binned ranks; one extra spare rank each)
RPLAN = {"lab": [0] * 5 + [1] * 5 + [2] * 5 + [3] * 5 + [4] * 5 + [5] * 3,
         "inj": [0] * 8 + [1] * 8 + [2] * 8 + [3] * 2}
NWIN = {"lab": 6, "inj": 4}
NBIN = {s: len(RPLAN[s]) for s in ("lab", "inj")}      # 28 / 26
NRANK = {s: NBIN[s] + 1 for s in ("lab", "inj")}       # incl spare: 29 / 27
CSW = {s: NRANK[s] * 128 for s in ("lab", "inj")}      # chunk-stream width
ITEM_OF = {"lab": "lab_item", "inj": "inj_item"}
VAL_OF = {"lab": "lab_value", "inj": "inj_value"}
# chunks whose VALUE rows come via SWDGE gather (on the otherwise-idle
# GpSimd) instead of one-hot matmuls, relieving PE/ACT/DVE
CONV = ()
# iota const layout: cols 0-1 value tiles, 2..7 lab windows, 8..11 inj windows
IOTA_I0 = {"lab": 2, "inj": 8}

WSHAPES = (("mwhhT", [128, 1920]), ("mwihT34", [128, 768]),
           ("mxgb34", [1, 768]), ("mwihT012", [128, 1152]),
           ("mxgb012", [1, 1152]), ("vwhhT", [128, 2688]),
           ("vwihT04", [128, 1920]), ("vxgb04", [1, 1920]),
           ("vxg56u", [1, 768]), ("vxg56c", [1, 768]),
           ("S5", [128, 5]), ("ones", [1, 224]),
           ("wa", [1, 64]), ("fcw", [128, 7 * OUT]), ("fcb", [1, OUT]),
           ("vtab_lab", [128, 256]), ("vtab_inj", [128, 256]),
           ("itab_lab", [128, NWIN["lab"] * 128]),
           ("itab_inj", [128, NWIN["inj"] * 128]))
WSHAPES_F32 = (("bhn_bc", [128, 160]), ("vbhn_bc", [128, 14]),
               ("iotas", [128, 12]))


# --------------------------------------------------------------------------
# host-side index / weight packing
# --------------------------------------------------------------------------

def _wrap_idx(flat):
    # token i lives at [i % 16, i // 16]; the gather ucode's Q7 cores each
    # read their own 16-partition band, so replicate to all 8 bands.
    n = flat.shape[0]
    return np.tile(flat.reshape(n // 16, 16).T, (8, 1)).astype(np.int16)


def _build_visit_idx(tok):
    flat = np.zeros(VIDX, dtype=np.int64)
    t = np.asarray(tok)
    for r in range(VRANKS):
        for j in range(5):
            slot = 5 * r + j
            if slot >= NBV:
                continue
            b, v = divmod(slot, V)
            flat[r * 128 + j * 24: r * 128 + j * 24 + 24] = t[b, v, :]
    return _wrap_idx(flat)


def _bin_chunk(ids_i, ids_v, grp, sname):
    """Bin one chunk-stream's tokens by item vocab window.

    ids_i/ids_v/grp: [NTOK] arrays (item id, value id, group id 0..127).
    Returns (ibc_row, vbc_row, S, spare_idx) where rows are [CSW] and
    S is [128, CSW] one-hot token->group.
    """
    nrank, nbin = NRANK[sname], NBIN[sname]
    W = CSW[sname]
    win = ids_i >> 7
    order = np.lexsort((grp, win))
    ids_i, ids_v, grp, win = (a[order] for a in (ids_i, ids_v, grp, win))
    plan = np.asarray(RPLAN[sname])
    ib = np.full(W, -1, dtype=np.int64)  # ids reduced by window base
    vb = np.zeros(W, dtype=np.int64)
    gr = np.full(W, -1, dtype=np.int64)
    spare = []
    pos = 0
    for w in range(NWIN[sname]):
        ranks = np.nonzero(plan == w)[0]
        cap = len(ranks) * 128
        base = ranks[0] * 128
        nw = int(np.searchsorted(win, w + 1)) - pos
        take = min(nw, cap)
        sl = slice(pos, pos + take)
        ib[base:base + take] = ids_i[sl] - 128 * w
        vb[base:base + take] = ids_v[sl]
        gr[base:base + take] = grp[sl]
        if nw > take:  # bin overflow -> spare rank
            spare.extend(range(pos + take, pos + nw))
        pos += nw
    assert len(spare) <= 128, f"spare overflow: {len(spare)}"
    sb = nbin * 128
    sp_idx = np.zeros(128, dtype=np.int64)
    if spare:
        sp = np.asarray(spare)
        n = len(sp)
        sp_idx[:n] = ids_i[sp]
        vb[sb:sb + n] = ids_v[sp]
        gr[sb:sb + n] = grp[sp]
    S = np.zeros((128, W), dtype=np.float32)
    cols = np.arange(W)
    real = gr >= 0
    S[cols[real] % 128, (cols[real] // 128) * 128 + gr[real]] = 1.0
    # S layout fix: S[p, r*128 + g]; p is the slot-within-rank
    return ib, vb, S, sp_idx


def _prep_shared(inputs):
    """Weight repacking shared by all cores (pure layout transforms)."""
    f = {k: np.asarray(v, dtype=np.float32) for k, v in inputs.items()
         if not k.startswith("tok_")}
    sh = {}
    mwih, mwhh = f["mgru_wih"], f["mgru_whh"]
    mbih, mbhh = f["mgru_bih"], f["mgru_bhh"]
    vwih, vwhh = f["vgru_wih"], f["vgru_whh"]
    vbih, vbhh = f["vgru_bih"], f["vgru_bhh"]

    def packT(w_keys):  # [K, 3D, D] -> [128, K*3*128], col (k*3+gi)*128+gu
        k = w_keys.shape[0]
        out = np.zeros((128, k * 3 * 128), dtype=np.float32)
        for ki in range(k):
            for gi in range(3):
                out[:, (ki * 3 + gi) * 128:(ki * 3 + gi + 1) * 128] = \
                    w_keys[ki, gi * 128:(gi + 1) * 128, :].T
        return out

    def pack_xgb(bih, bhh, keys):  # -> [1, len(keys)*384]
        rows = []
        for k in keys:
            b = bih[k].copy()
            b[:2 * D] += bhh[k][:2 * D]
            rows.append(b)
        return np.concatenate(rows)[None, :]

    sh["mwhhT"] = packT(mwhh)
    sh["mwihT34"] = packT(mwih[3:5])
    sh["mxgb34"] = pack_xgb(mbih, mbhh, [3, 4])
    sh["mwihT012"] = packT(mwih[0:3])
    sh["mxgb012"] = pack_xgb(mbih, mbhh, [0, 1, 2])
    sh["bhn_bc"] = np.repeat(mbhh[:, 2 * D:].T, NBV, axis=1).astype(np.float32)
    sh["vwhhT"] = packT(vwhh)
    sh["vwihT04"] = packT(vwih[0:5])
    sh["vxgb04"] = pack_xgb(vbih, vbhh, [0, 1, 2, 3, 4])
    u_rows, c_rows = [], []
    for k in (5, 6):
        u_rows.append(vwih[k] @ f["info_w"][k - 5])
        cv = vwih[k] @ f["info_b"][k - 5] + vbih[k]
        cv[:2 * D] += vbhh[k][:2 * D]
        c_rows.append(cv)
    sh["vxg56u"] = np.concatenate(u_rows)[None, :]
    sh["vxg56c"] = np.concatenate(c_rows)[None, :]
    sh["vbhn_bc"] = np.repeat(vbhh[:, 2 * D:].T, BL, axis=1).astype(np.float32)
    s5 = np.zeros((128, 5), dtype=np.float32)
    for j in range(5):
        s5[j * 24:(j + 1) * 24, j] = 1.0
    sh["S5"] = s5
    sh["ones"] = np.ones((1, 224), dtype=np.float32)
    fcw = np.zeros((128, 7 * OUT), dtype=np.float32)
    for k in range(7):
        fcw[:, k * OUT:(k + 1) * OUT] = f["fc_w"][k * D:(k + 1) * D, :]
    sh["fcw"] = fcw
    sh["fcb"] = f["fc_b"][None, :]
    for s in ("lab", "inj"):
        tabf = f["emb_" + VAL_OF[s]]
        vt = np.zeros((128, 256), dtype=np.float32)
        vt[:, 0:128] = tabf[0:128]
        vt[:tabf.shape[0] - 128, 128:256] = tabf[128:]
        sh["vtab_" + s] = vt
        tabi = f["emb_" + ITEM_OF[s]]
        it = np.zeros((128, NWIN[s] * 128), dtype=np.float32)
        for w in range(NWIN[s]):
            rows = tabi[w * 128:(w + 1) * 128]
            it[:rows.shape[0], w * 128:w * 128 + 128] = rows
        sh["itab_" + s] = it
    iv = np.zeros((128, 12), dtype=np.float32)
    p = np.arange(128)
    iv[:, 0] = p
    iv[:, 1] = p + 128
    for s in ("lab", "inj"):
        for w in range(NWIN[s]):
            iv[:, IOTA_I0[s] + w] = p + 128 * w
    sh["iotas"] = iv
    out = {}
    for n, _ in WSHAPES:
        if n == "wa":
            continue
        out[n] = sh[n].astype(BF)
    for n, _ in WSHAPES_F32:
        out[n] = sh[n].astype(np.float32)
    for name in ("cond", "proc", "drug", "lab_item", "inj_item",
                 "lab_value", "inj_value"):
        out["emb_" + name] = f["emb_" + name].astype(BF)
    return out


def _prep_core(inputs, shared, core):
    b0 = core * BL
    m = dict(shared)
    for name in ("cond", "proc", "drug"):
        m["idx_" + name] = _build_visit_idx(
            np.asarray(inputs["tok_" + name])[b0:b0 + BL])
    for sname in ("lab", "inj"):
        ti = np.asarray(inputs["tok_" + ITEM_OF[sname]])[b0:b0 + BL] \
            .reshape(NBV, M, L)
        tv = np.asarray(inputs["tok_" + VAL_OF[sname]])[b0:b0 + BL] \
            .reshape(NBV, M, L)
        sp_flat = np.zeros(NCHUNK * 128, dtype=np.int64)
        for c in range(NCHUNK):
            msl = slice(c * TCH, (c + 1) * TCH)
            ids_i = ti[:, msl, :].transpose(1, 0, 2).reshape(-1)
            ids_v = tv[:, msl, :].transpose(1, 0, 2).reshape(-1)
            grp = (np.arange(TCH)[:, None, None] * NBV +
                   np.arange(NBV)[None, :, None] +
                   0 * np.arange(L)[None, None, :]).reshape(-1)
            ib, vb, S, sp = _bin_chunk(ids_i, ids_v, grp, sname)
            m[f"ibc_{sname}_{c}"] = np.ascontiguousarray(
                np.broadcast_to(ib.astype(BF)[None, :], (128, CSW[sname])))
            if (c, sname) in CONV:
                m[f"vidx_{sname}_{c}"] = _wrap_idx(vb)
            else:
                m[f"vbc_{sname}_{c}"] = np.ascontiguousarray(
                    np.broadcast_to(vb.astype(BF)[None, :], (128, CSW[sname])))
            m[f"smat_{sname}_{c}"] = S.astype(F8NP)
            sp_flat[c * 128:(c + 1) * 128] = sp
        m[f"spidx_{sname}"] = _wrap_idx(sp_flat)
    wa = np.zeros((1, 64), dtype=np.float32)
    wa[0, :NBV] = np.asarray(inputs["weight"], np.float32)[b0:b0 + BL].reshape(NBV)
    wa[0, NBV:] = np.asarray(inputs["age"], np.float32)[b0:b0 + BL].reshape(NBV)
    m["wa"] = wa.astype(BF)
    return m


# --------------------------------------------------------------------------
# device program
# --------------------------------------------------------------------------

def build_nc():
    nc = bacc.Bacc("TRN2", target_bir_lowering=False, debug=False,
                   num_devices=NCORES)
    dt = {}
    for name in ("cond", "proc", "drug", "lab_item", "inj_item",
                 "lab_value", "inj_value"):
        dt["emb_" + name] = nc.dram_tensor("emb_" + name, [VOCAB[name], D],
                                           BF16, kind="ExternalInput")
    for name in ("cond", "proc", "drug"):
        dt["idx_" + name] = nc.dram_tensor("idx_" + name, [128, VIDX // 16],
                                           I16, kind="ExternalInput")
    for s in ("lab", "inj"):
        dt[f"spidx_{s}"] = nc.dram_tensor(f"spidx_{s}", [128, NCHUNK * 8],
                                          I16, kind="ExternalInput")
        for c in range(NCHUNK):
            dt[f"ibc_{s}_{c}"] = nc.dram_tensor(f"ibc_{s}_{c}",
                                                [128, CSW[s]], BF16,
                                                kind="ExternalInput")
            if (c, s) in CONV:
                dt[f"vidx_{s}_{c}"] = nc.dram_tensor(
                    f"vidx_{s}_{c}", [128, CSW[s] // 16], I16,
                    kind="ExternalInput")
            else:
                dt[f"vbc_{s}_{c}"] = nc.dram_tensor(f"vbc_{s}_{c}",
                                                    [128, CSW[s]], BF16,
                                                    kind="ExternalInput")
            dt[f"smat_{s}_{c}"] = nc.dram_tensor(f"smat_{s}_{c}",
                                                 [128, CSW[s]], F8,
                                                 kind="ExternalInput")
    for name, shape in WSHAPES:
        dt[name] = nc.dram_tensor(name, shape, BF16, kind="ExternalInput")
    for name, shape in WSHAPES_F32:
        dt[name] = nc.dram_tensor(name, shape, F32, kind="ExternalInput")
    out_logits = nc.dram_tensor("logits", [BL, OUT], F32, kind="ExternalOutput")

    with tile.TileContext(nc) as tc:
        _program(nc, tc, dt, out_logits)
    nc.compile()
    return nc


def _program(nc, tc, dt, out_logits):
    import contextlib
    ctx = contextlib.ExitStack()
    with ctx:
        cpool = ctx.enter_context(tc.tile_pool(name="const", bufs=1))
        gpool = ctx.enter_context(tc.tile_pool(name="gather", bufs=2))
        vbpool = ctx.enter_context(tc.tile_pool(name="vb", bufs=2))
        ohpool = ctx.enter_context(tc.tile_pool(name="oh", bufs=2))
        prpool = ctx.enter_context(tc.tile_pool(name="pr", bufs=4))
        spool = ctx.enter_context(tc.tile_pool(name="work", bufs=2))
        xgpool = ctx.enter_context(tc.tile_pool(name="xg34", bufs=3))
        hpool = ctx.enter_context(tc.tile_pool(name="h", bufs=2))
        p2 = ctx.enter_context(tc.tile_pool(name="psum2", bufs=2,
                                            space="PSUM"))
        p1 = ctx.enter_context(tc.tile_pool(name="psum1", bufs=1,
                                            space="PSUM"))

        # ---- load constants to SBUF; critical-path ones first, then chunk-0
        # stream data, then the rest (the sync queue issues in order).
        shapes = dict(WSHAPES)
        shapes_f32 = dict(WSHAPES_F32)
        cb = {}

        def _load(name):
            if name in shapes_f32:
                t = cpool.tile(shapes_f32[name], F32, tag=name)
            else:
                t = cpool.tile(shapes[name], BF16, tag=name)
            nc.sync.dma_start(t[:], dt[name].ap())
            cb[name] = t

        idx = {}
        for name in ("cond", "proc", "drug"):
            t = cpool.tile([128, VIDX // 16], I16, tag="idx_" + name)
            nc.sync.dma_start(t[:], dt["idx_" + name].ap())
            idx[name] = t
        for name in ("iotas", "vtab_lab", "vtab_inj", "itab_lab", "itab_inj",
                     "S5", "ones", "mwihT012", "mxgb012", "mwhhT", "bhn_bc",
                     "mwihT34", "mxgb34", "vxg56u", "vxg56c", "wa", "vwhhT",
                     "vbhn_bc"):
            _load(name)

        # chunk-stream tile prefetch (vb/ib/st for chunk c)
        stream_tiles = {}

        def _prefetch(c):
            if c >= NCHUNK:
                return
            for sname in ("lab", "inj"):
                W = CSW[sname]
                if (c, sname) in CONV:
                    vb = None
                else:
                    vb = vbpool.tile([128, W], BF16, tag="vb_" + sname)
                    nc.sync.dma_start(vb[:], dt[f"vbc_{sname}_{c}"].ap())
                ib = vbpool.tile([128, W], BF16, tag="ib_" + sname)
                nc.sync.dma_start(ib[:], dt[f"ibc_{sname}_{c}"].ap())
                st = vbpool.tile([128, W], F8, tag="st_" + sname)
                nc.sync.dma_start(st[:], dt[f"smat_{sname}_{c}"].ap())
                stream_tiles[(c, sname)] = (vb, ib, st)

        _prefetch(0)
        spi = {}
        for s in ("lab", "inj"):
            t = cpool.tile([128, NCHUNK * 8], I16, tag="spidx_" + s)
            nc.sync.dma_start(t[:], dt[f"spidx_{s}"].ap())
            spi[s] = t
        for name in ("vwihT04", "vxgb04", "fcw", "fcb"):
            _load(name)

        S5 = cb["S5"]
        ones = cb["ones"]
        iotas = cb["iotas"]

        # ---- visit-level features eT[k] = [128, 32] bf16
        eT = {}
        for name in ("cond", "proc", "drug"):
            gt = gpool.tile([128, VIDX], BF16, tag="vgather")
            nc.gpsimd.dma_gather(
                gt[:].rearrange("p (r e) -> p r e", e=D),
                dt["emb_" + name].ap(), idx[name][:], VIDX, VIDX, D)
            pr = p1.tile([128, 128], F32, tag="red")
            for r in range(VRANKS):
                nc.tensor.matmul(pr[:, 5 * r:5 * r + 5],
                                 gt[:, r * D:(r + 1) * D], S5[:],
                                 start=True, stop=True)
            et = cpool.tile([128, NBV], BF16, tag="eT_" + name)
            nc.scalar.copy(et[:], pr[:, :NBV])
            eT[name] = et

        # ---- spare-rank item gathers (one per stream, all chunks)
        spg = {}
        for s in ("lab", "inj"):
            gt = cpool.tile([128, NCHUNK * 128], BF16, tag="spg_" + s)
            nc.gpsimd.dma_gather(
                gt[:].rearrange("p (r e) -> p r e", e=D),
                dt["emb_" + ITEM_OF[s]].ap(), spi[s][:],
                NCHUNK * 128, NCHUNK * 128, D)
            spg[s] = gt

        # ---- value-row gathers for the CONV chunks run just-in-time on the
        # otherwise-idle GpSimd, two chunks ahead of use
        convt = {}
        vix = {}
        for cc, s in CONV:
            t = cpool.tile([128, CSW[s] // 16], I16, tag=f"vidx_{s}_{cc}")
            nc.sync.dma_start(t[:], dt[f"vidx_{s}_{cc}"].ap())
            vix[(cc, s)] = t

        def _conv_gather(cc):
            for s in ("lab", "inj"):
                if (cc, s) not in CONV:
                    continue
                W = CSW[s]
                vt = vbpool.tile([128, W], BF16, tag="vt_" + s)
                for r0 in range(0, NRANK[s], 8):
                    nr = min(8, NRANK[s] - r0)
                    nc.gpsimd.dma_gather(
                        vt[:].rearrange("p (r e) -> p r e", e=D)
                        [:, r0:r0 + nr, :],
                        dt["emb_" + VAL_OF[s]].ap(),
                        vix[(cc, s)][:, r0 * 8:(r0 + nr) * 8],
                        nr * 128, nr * 128, D)
                convt[(cc, s)] = vt

        # ---- XGc for monitor keys 0-2: [128, 3*96], col gi*96 + k*32 + bv
        xgc = cpool.tile([128, 288], BF16, tag="xgc")
        for k, name in enumerate(("cond", "proc", "drug")):
            pk = p1.tile([128, 384], F32, tag="xg")
            for gi in range(3):
                off = (k * 3 + gi) * 128
                nc.tensor.matmul(pk[:, gi * 32:(gi + 1) * 32],
                                 cb["mwihT012"][:, off:off + 128],
                                 eT[name][:], start=True, stop=False)
                nc.tensor.matmul(pk[:, gi * 32:(gi + 1) * 32],
                                 cb["mxgb012"][0:1, off:off + 128],
                                 ones[0:1, 0:NBV], start=False, stop=True)
            nc.scalar.copy(
                xgc[:].rearrange("p (g c) -> p g c", g=3)[:, :, k * 32:(k + 1) * 32],
                pk[:, 0:96].rearrange("p (g c) -> p g c", g=3))

        # ---- visit GRU xg for keys 5,6 (weight/age)
        vxg = cpool.tile([128, 672], BF16, tag="vxg")  # col gi*224 + k*32 + bv
        for k in (5, 6):
            pk = p1.tile([128, 384], F32, tag="xg")
            for gi in range(3):
                off = ((k - 5) * 3 + gi) * 128
                nc.tensor.matmul(pk[:, gi * 32:(gi + 1) * 32],
                                 cb["vxg56u"][0:1, off:off + 128],
                                 cb["wa"][0:1, (k - 5) * 32:(k - 4) * 32],
                                 start=True, stop=False)
                nc.tensor.matmul(pk[:, gi * 32:(gi + 1) * 32],
                                 cb["vxg56c"][0:1, off:off + 128],
                                 ones[0:1, 0:NBV], start=False, stop=True)
            nc.scalar.copy(
                vxg[:].rearrange("p (g c) -> p g c", g=3)[:, :, k * 32:(k + 1) * 32],
                pk[:, 0:96].rearrange("p (g c) -> p g c", g=3))

        # ---- visit GRU keys 5,6 final state (steps interleaved into chunks)
        vh56 = cpool.tile([128, 4], BF16, tag="vh56")

        # ---- monitor chunks + GRU
        h_prev = hpool.tile([128, 160], BF16, tag="H")
        nc.vector.memset(h_prev[:], 0.0)

        vh0 = hpool.tile([128, 4], BF16, tag="VH56")
        nc.vector.memset(vh0[:], 0.0)
        vh56_state = [vh0]

        def emit_vh56(v):
            # one keys-5/6 visit-GRU step, hidden in chunk slack
            pz = p2.tile([128, 42], F32, tag="v56")
            rz = spool.tile([128, 28], BF16, tag="vrz")
            u = spool.tile([128, 14], BF16, tag="vu")
            nt = spool.tile([128, 14], BF16, tag="vnt")
            t3 = spool.tile([128, 14], BF16, tag="vt3")
            vh_new = vh56 if v == V - 1 else hpool.tile([128, 4], BF16,
                                                        tag="VH56")
            _vgru_sub(nc, cb, vxg, pz[:, 0:28], pz[:, 28:42], rz, u, nt, t3,
                      vh56_state[0], vh_new, v, 5, 7, hofs=10)
            vh56_state[0] = vh_new

        for c in range(NCHUNK):
            xg34 = xgpool.tile([128, 768], BF16, tag="xg34")
            _prefetch(c + 1)
            if c + 4 in [cc for cc, _ in CONV]:
                _conv_gather(c + 4)
            emit_vh56(2 * c)
            for k, sname in ((3, "lab"), (4, "inj")):
                W = CSW[sname]
                nbin, nrank = NBIN[sname], NRANK[sname]
                vb, ib, st = stream_tiles.pop((c, sname))
                conv = (c, sname) in CONV
                vtab, itab = cb["vtab_" + sname], cb["itab_" + sname]
                ohi = ohpool.tile([128, W], BF16, tag="ohi")
                if not conv:
                    oh0 = ohpool.tile([128, W], BF16, tag="oh0")
                    oh1 = ohpool.tile([128, W], BF16, tag="oh1")
                    nc.vector.tensor_scalar(oh0[:], vb[:], iotas[:, 0:1],
                                            None, OP.is_equal)
                    nc.vector.tensor_scalar(oh1[:], vb[:], iotas[:, 1:2],
                                            None, OP.is_equal)
                plan = RPLAN[sname]
                nc.vector.tensor_scalar(
                    ohi[:, 0:nbin * 128], ib[:, 0:nbin * 128],
                    iotas[:, 0:1], None, OP.is_equal)
                xsum = p1.tile([128, 128], F32, tag="red")
                s_pending = []
                for r0 in range(0, nrank, 4):
                    nr = min(4, nrank - r0)
                    wd = nr * 128
                    if conv:
                        vtile, vofs = convt[(c, sname)], r0 * 128
                    else:
                        pv = p2.tile([128, 512], F32, tag="pv")
                        for j in range(nr):
                            sl = slice((r0 + j) * 128, (r0 + j + 1) * 128)
                            nc.tensor.matmul(pv[:, j * 128:(j + 1) * 128],
                                             oh0[:, sl], vtab[:, 0:128],
                                             start=True, stop=False)
                            nc.tensor.matmul(pv[:, j * 128:(j + 1) * 128],
                                             oh1[:, sl], vtab[:, 128:256],
                                             start=False, stop=True)
                        vtile = prpool.tile([128, 512], BF16, tag="pvb")
                        nc.scalar.copy(vtile[:, 0:wd], pv[:, 0:wd])
                        vofs = 0
                    pt = prpool.tile([128, 512], BF16, tag="prod")
                    nbin_j = min(nr, nbin - r0)  # binned ranks in this group
                    if nbin_j > 0:
                        pi = p2.tile([128, 512], F32, tag="pv")
                        for j in range(nbin_j):
                            w = plan[r0 + j]
                            sl = slice((r0 + j) * 128, (r0 + j + 1) * 128)
                            nc.tensor.matmul(
                                pi[:, j * 128:(j + 1) * 128],
                                ohi[:, sl], itab[:, w * 128:(w + 1) * 128],
                                start=True, stop=True)
                        nc.vector.tensor_tensor(pt[:, 0:nbin_j * 128],
                                                pi[:, 0:nbin_j * 128],
                                                vtile[:, vofs:vofs + nbin_j * 128],
                                                op=OP.mult)
                    if nbin_j < nr:  # spare rank (always last)
                        j = nbin_j
                        nc.vector.tensor_tensor(
                            pt[:, j * 128:(j + 1) * 128],
                            spg[sname][:, c * 128:(c + 1) * 128],
                            vtile[:, vofs + j * 128:vofs + (j + 1) * 128],
                            op=OP.mult)
                    for args in s_pending:
                        nc.tensor.matmul(*args[:3], start=args[3], stop=args[4],
                                         skip_group_check=True)
                    s_pending = []
                    for j in range(nr):
                        r = r0 + j
                        s_pending.append(
                            (xsum[:], pt[:, j * 128:(j + 1) * 128],
                             st[:, r * 128:(r + 1) * 128],
                             r == 0, r == nrank - 1))
                for args in s_pending:
                    nc.tensor.matmul(*args[:3], start=args[3], stop=args[4],
                                     skip_group_check=True)
                xmT = spool.tile([128, GC], BF16, tag="xmT")
                nc.scalar.copy(xmT[:], xsum[:])
                pxg = p1.tile([128, 384], F32, tag="xg")
                for gi in range(3):
                    off = ((k - 3) * 3 + gi) * 128
                    nc.tensor.matmul(pxg[:, gi * 128:(gi + 1) * 128],
                                     cb["mwihT34"][:, off:off + 128],
                                     xmT[:], start=True, stop=False)
                    nc.tensor.matmul(pxg[:, gi * 128:(gi + 1) * 128],
                                     cb["mxgb34"][0:1, off:off + 128],
                                     ones[0:1, 0:128], start=False, stop=True)
                # copy into xg34: col gi*256 + mi*64 + (k-3)*32 + bv
                for gi in range(3):
                    nc.scalar.copy(
                        xg34[:].rearrange("p (g m kb) -> p g m kb", g=3, m=TCH)
                        [:, gi, :, (k - 3) * 32:(k - 2) * 32],
                        pxg[:].rearrange("p (g m b) -> p g m b", g=3, m=TCH)
                        [:, gi, :, :])
                if sname == "lab":
                    emit_vh56(2 * c + 1)

            for mi in range(TCH):
                gt_ = p2.tile([128, 480], F32, tag="gru")
                prz = gt_[:, 0:320]
                pn = gt_[:, 320:480]
                rz = spool.tile([128, 320], BF16, tag="rz")
                npre = spool.tile([128, 160], BF16, tag="npre")
                nt = spool.tile([128, 160], BF16, tag="nt")
                t3 = spool.tile([128, 160], BF16, tag="t3")
                h_new = hpool.tile([128, 160], BF16, tag="H")
                # two key-groups: A = keys 0-2 (cols 0:96), B = keys 3-4
                for klo, khi in ((0, 3), (3, 5)):
                    lo, hi = klo * 32, khi * 32
                    if klo == 0:
                        nc.scalar.copy(
                            prz.rearrange("p (g c) -> p g c", g=2)[:, :, 0:96],
                            xgc[:].rearrange("p (g c) -> p g c", g=3)[:, 0:2, :])
                    else:
                        nc.scalar.copy(
                            prz.rearrange("p (g c) -> p g c", g=2)[:, :, 96:160],
                            xg34[:].rearrange("p (g c) -> p g c", g=3)
                            [:, 0:2, mi * 64:(mi + 1) * 64])
                    nc.scalar.copy(pn[:, lo:hi], cb["bhn_bc"][:, lo:hi])
                    for k in range(klo, khi):
                        hs = h_prev[:, k * 32:(k + 1) * 32]
                        for gi in range(2):
                            nc.tensor.matmul(
                                prz[:, gi * 160 + k * 32: gi * 160 + (k + 1) * 32],
                                cb["mwhhT"][:, (k * 3 + gi) * 128:(k * 3 + gi + 1) * 128],
                                hs, start=False, stop=True, skip_group_check=True)
                        nc.tensor.matmul(
                            pn[:, k * 32:(k + 1) * 32],
                            cb["mwhhT"][:, (k * 3 + 2) * 128:(k * 3 + 3) * 128],
                            hs, start=False, stop=True, skip_group_check=True)
                    nc.scalar.activation(
                        rz[:].rearrange("p (g c) -> p g c", g=2)[:, :, lo:hi],
                        prz.rearrange("p (g c) -> p g c", g=2)[:, :, lo:hi],
                        AF.Sigmoid)
                    nc.vector.tensor_tensor(npre[:, lo:hi], rz[:, lo:hi],
                                            pn[:, lo:hi], op=OP.mult)
                    if klo == 0:
                        nc.vector.tensor_tensor(
                            npre[:, lo:hi], npre[:, lo:hi],
                            xgc[:].rearrange("p (g c) -> p g c", g=3)[:, 2, :],
                            op=OP.add)
                    else:
                        nc.vector.tensor_tensor(
                            npre[:, lo:hi], npre[:, lo:hi],
                            xg34[:, 2 * 256 + mi * 64: 2 * 256 + (mi + 1) * 64],
                            op=OP.add)
                    nc.scalar.activation(nt[:, lo:hi], npre[:, lo:hi], AF.Tanh)
                    nc.vector.tensor_tensor(t3[:, lo:hi], h_prev[:, lo:hi],
                                            nt[:, lo:hi], op=OP.subtract)
                    nc.vector.tensor_tensor(t3[:, lo:hi], t3[:, lo:hi],
                                            rz[:, 160 + lo:160 + hi],
                                            op=OP.mult)
                    nc.vector.tensor_tensor(h_new[:, lo:hi], t3[:, lo:hi],
                                            nt[:, lo:hi], op=OP.add)
                h_prev = h_new



        # ---- visit GRU xg for keys 0-4 (from final monitor h)
        for k in range(5):
            pk = p1.tile([128, 384], F32, tag="xg")
            for gi in range(3):
                off = (k * 3 + gi) * 128
                nc.tensor.matmul(pk[:, gi * 32:(gi + 1) * 32],
                                 cb["vwihT04"][:, off:off + 128],
                                 h_prev[:, k * 32:(k + 1) * 32],
                                 start=True, stop=False)
                nc.tensor.matmul(pk[:, gi * 32:(gi + 1) * 32],
                                 cb["vxgb04"][0:1, off:off + 128],
                                 ones[0:1, 0:NBV], start=False, stop=True)
            nc.scalar.copy(
                vxg[:].rearrange("p (g c) -> p g c", g=3)[:, :, k * 32:(k + 1) * 32],
                pk[:, 0:96].rearrange("p (g c) -> p g c", g=3))

        # ---- visit GRU keys 0-4 (16 steps)
        vh_prev = hpool.tile([128, 10], BF16, tag="VH")
        nc.vector.memset(vh_prev[:], 0.0)
        for v in range(V):
            gt_ = p2.tile([128, 480], F32, tag="gru")
            rz = spool.tile([128, 28], BF16, tag="vrz")
            u = spool.tile([128, 14], BF16, tag="vu")
            nt = spool.tile([128, 14], BF16, tag="vnt")
            t3 = spool.tile([128, 14], BF16, tag="vt3")
            vh_new = hpool.tile([128, 10], BF16, tag="VH")
            for klo, khi in ((0, 2), (2, 5)):
                _vgru_sub(nc, cb, vxg, gt_[:, 0:320], gt_[:, 320:480],
                          rz, u, nt, t3, vh_prev, vh_new, v, klo, khi, hofs=0)
            vh_prev = vh_new

        # ---- FC head
        rl = spool.tile([128, 14], BF16, tag="rl")
        nc.scalar.activation(rl[:, 0:10], vh_prev[:], AF.Relu)
        nc.scalar.activation(rl[:, 10:14], vh56[:], AF.Relu)
        pfc = p1.tile([BL, 384], F32, tag="xg")
        for k in range(7):
            nc.tensor.matmul(pfc[:, 0:OUT], rl[:, k * 2:(k + 1) * 2],
                             cb["fcw"][:, k * OUT:(k + 1) * OUT],
                             start=(k == 0), stop=False)
        nc.tensor.matmul(pfc[:, 0:OUT], ones[0:1, 0:BL], cb["fcb"][0:1, :],
                         start=False, stop=True)
        lg = spool.tile([BL, OUT], F32, tag="lg")
        nc.scalar.copy(lg[:], pfc[:, 0:OUT])
        nc.sync.dma_start(out_logits.ap(), lg[:])


def _vgru_sub(nc, cb, vxg, prz, pn, rz, u, nt, t3, vh_prev, vh_new,
              v, klo, khi, hofs):
    """One key-subrange of one visit-GRU step.

    vh_prev/vh_new are [128, 2*(khi-klo...)] slices indexed relative to hofs:
    h column for key k lives at (k*2 - hofs*2//...) -- callers pass tiles
    whose col 0 corresponds to key hofs//2... simplified: vh tiles hold keys
    [hofs/2, ...) with col (k - hofs_key)*2.
    """
    kofs = hofs // 2  # first key held in the vh tiles
    lo, hi = klo * 2, khi * 2
    nc.vector.tensor_scalar(
        prz[:, 0:28].rearrange("p (g c) -> p g c", g=2)[:, :, lo:hi],
        vxg[:].rearrange("p (g k b v2) -> p g k b v2", g=3, k=7, b=BL)
        [:, 0:2, klo:khi, :, v], 0.0, None, OP.add)
    nc.vector.tensor_scalar(pn[:, lo:hi], cb["vbhn_bc"][:, lo:hi],
                            0.0, None, OP.add)
    for k in range(klo, khi):
        hs = vh_prev[:, (k - kofs) * 2:(k - kofs + 1) * 2]
        for gi in range(2):
            nc.tensor.matmul(
                prz[:, gi * 14 + k * 2: gi * 14 + (k + 1) * 2],
                cb["vwhhT"][:, (k * 3 + gi) * 128:(k * 3 + gi + 1) * 128],
                hs, start=False, stop=True, skip_group_check=True)
        nc.tensor.matmul(
            pn[:, k * 2:(k + 1) * 2],
            cb["vwhhT"][:, (k * 3 + 2) * 128:(k * 3 + 3) * 128],
            hs, start=False, stop=True, skip_group_check=True)
    nc.scalar.activation(
        rz[:].rearrange("p (g c) -> p g c", g=2)[:, :, lo:hi],
        prz[:, 0:28].rearrange("p (g c) -> p g c", g=2)[:, :, lo:hi],
        AF.Sigmoid)
    nc.vector.tensor_tensor(u[:, lo:hi], rz[:, lo:hi], pn[:, lo:hi],
                            op=OP.mult)
    nc.vector.tensor_tensor(
        u[:, lo:hi], u[:, lo:hi],
        vxg[:].rearrange("p (g k b v2) -> p g k b v2", g=3, k=7, b=BL)
        [:, 2, klo:khi, :, v],
        op=OP.add)
    nc.scalar.activation(nt[:, lo:hi], u[:, lo:hi], AF.Tanh)
    hsl = slice(lo - hofs, hi - hofs)
    nc.vector.tensor_tensor(t3[:, lo:hi], vh_prev[:, hsl], nt[:, lo:hi],
                            op=OP.subtract)
    nc.vector.tensor_tensor(t3[:, lo:hi], t3[:, lo:hi],
                            rz[:, 14 + lo:14 + hi], op=OP.mult)
    nc.vector.tensor_tensor(vh_new[:, hsl], t3[:, lo:hi], nt[:, lo:hi],
                            op=OP.add)


# --------------------------------------------------------------------------
# entry point
# --------------------------------------------------------------------------

_NC_CACHE = None


def kernel(**inputs):
    global _NC_CACHE
    if _NC_CACHE is None:
        _NC_CACHE = build_nc()
    nc = _NC_CACHE
    shared = _prep_shared(inputs)
    in_maps = [_prep_core(inputs, shared, c) for c in range(NCORES)]
    res = run_bass_kernel_spmd(nc, in_maps, core_ids=list(range(NCORES)))
    return np.concatenate([res.results[c]["logits"] for c in range(NCORES)],
                          axis=0).astype(np.float32)


if __name__ == "__main__":
    import reference
    inputs = {k: np.asarray(v) for k, v in reference.setup_inputs().items()}
    out = kernel(**inputs)
    print("out", out.shape, out.dtype)
